# revision 8
# baseline (speedup 1.0000x reference)
"""Trainium2 Bass kernel for nn_Conv1dMapper (3x conv1d+bn -> 3x fc+bn -> interp epilogue).

Self-contained: accepts FULL inputs, shards across 8 NeuronCores internally,
returns the FULL [64, 12, 100] output.

Sharding strategy (v3, weights-moving):
  - conv stage (tiny) replicated on all cores in bf16; bn1/bn2 folded into the
    next conv's weights at runtime; conv3 packs even/odd output positions into
    128 partitions via PE column-tiling so fc1 gets K=128 contraction chunks.
  - ALL fc matmuls are "weights-moving": the activations (batch=64 wide) are
    the stationary operand and the weights stream through the PE array with a
    512-wide free dim.  This turns each fc layer into ~N_chunks*3 large
    matmuls instead of thousands of N=64 ones.
  - fc1 row-sharded (1250 rows/core).  Output lands as [batch, feat] in PSUM;
    PE-transpose (identity matmul) flips each 125-chunk back to [feat, batch],
    where bias+relu+bn4 run exactly like a weights-stationary kernel.
  - the bf16 bn4 output (160 KB) is AllGather'd so every core holds the full
    10000-dim h1; fc2 is then row-sharded too (1250 outs/core, full
    contraction) - no ReduceScatter of fp32 partials needed.
  - fc3 contraction-sharded with the output epilogue folded into its weights;
    fp32 partials [64, 1200] are returned per-core and summed on the host.
"""

import sys

sys.path.insert(0, "/opt/trn_rl_repo")

import numpy as np

N_CORES = 8
B = 64            # batch
L1, L2, L3 = 98, 96, 94
NCH = 64          # conv channels
H = 6016          # fc1 in features = 64*94
L3H = 47          # = L3 // 2
HID = 10000
PREAL = 1250      # fc1/fc2 output rows per core
EPS = 1e-5
OUTF = 1200

K1 = 47           # fc1 contraction chunks (128 each)
J2 = 80           # fc2 contraction chunks (125 each)
FT = [(0, 512), (512, 512), (1024, 226)]    # f-tiles over 1250 local outs
FT3 = [(0, 512), (512, 512), (1024, 176)]   # f-tiles over 1200 outs
G1 = [8, 8, 8, 8, 8, 7]                     # fc1 kk-chunk DMA groups
G2N = 10                                    # fc2 j-chunk DMA groups (8 each)

_CACHE = {}


# ---------------------------------------------------------------- host prep

def _fold_epilogue(fc3_w, fc3_b):
    """Fold reshape->zero/one channels->interpolation into fc3's weights."""
    L = 100
    CD = L // 3
    CPS = np.array([1, CD, 2 * CD, 3 * CD])
    REG = np.array([p for p in range(L) if p not in set(CPS.tolist())][1:])
    J = REG // CD
    Lp = CPS[J]
    Rp = CPS[J + 1]
    ALPHA = ((REG - Lp) / CD).astype(np.float32)
    CH = np.array([0, 2, 8, 10, 3, 11])
    ZERO_CH = np.array([1, 4, 6, 7, 9])

    W3e = fc3_w.astype(np.float32).copy()
    b3e = fc3_b.astype(np.float32).copy()
    idx0 = (ZERO_CH[:, None] * L + np.arange(L)[None, :]).ravel()
    W3e[idx0] = 0.0
    b3e[idx0] = 0.0
    idx1 = 5 * L + np.arange(L)
    W3e[idx1] = 0.0
    b3e[idx1] = 1.0
    rows_t = (CH[:, None] * L + REG[None, :]).ravel()
    rows_l = (CH[:, None] * L + Lp[None, :]).ravel()
    rows_r = (CH[:, None] * L + Rp[None, :]).ravel()
    a = np.broadcast_to(ALPHA[None, :], (len(CH), len(REG))).ravel()[:, None]
    W3e[rows_t] = a * fc3_w[rows_l] + (1.0 - a) * fc3_w[rows_r]
    b3e[rows_t] = (a[:, 0] * fc3_b[rows_l] + (1.0 - a[:, 0]) * fc3_b[rows_r])
    return W3e, b3e


def _prep_in_maps(inp):
    from concourse import mybir

    f32 = np.float32
    bf16 = mybir.dt.np(mybir.dt.bfloat16)
    x = np.asarray(inp["x"], f32)

    # conv1 im2col: X9[k*3+i, l*64+b] = x[b, i, l+k]
    x_t = np.ascontiguousarray(x.transpose(1, 2, 0))      # [3, 100, 64]
    X9 = np.stack([x_t[:, k:k + L1, :] for k in range(3)], 0)  # [k, i, l, b]
    X9 = X9.reshape(9, L1 * B)
    X9 = np.ascontiguousarray(
        np.concatenate([X9, np.zeros((23, L1 * B), f32)], 0)).astype(bf16)

    w1 = np.asarray(inp["conv1_w"], f32).transpose(2, 1, 0).reshape(9, NCH)
    w1 = np.ascontiguousarray(
        np.concatenate([w1, np.zeros((23, NCH), f32)], 0))
    w1h = w1.astype(bf16)
    w1l = (w1 - w1h.astype(f32)).astype(bf16)
    w2 = np.ascontiguousarray(
        np.asarray(inp["conv2_w"], f32).transpose(1, 2, 0).reshape(NCH, 3 * NCH))
    w3 = np.ascontiguousarray(
        np.asarray(inp["conv3_w"], f32).transpose(1, 2, 0).reshape(NCH, 3 * NCH))

    common = {
        "X9": X9, "w1h": w1h, "w1l": w1l, "w2": w2, "w3": w3,
        "cb1": np.asarray(inp["conv1_b"], f32),
        "cb2": np.asarray(inp["conv2_b"], f32),
        "cb3": np.asarray(inp["conv3_b"], f32),
        "g1": np.asarray(inp["bn1_g"], f32), "be1": np.asarray(inp["bn1_b"], f32),
        "g2": np.asarray(inp["bn2_g"], f32), "be2": np.asarray(inp["bn2_b"], f32),
        "g3": np.asarray(inp["bn3_g"], f32), "be3": np.asarray(inp["bn3_b"], f32),
        "EYE": np.eye(B, dtype=f32),
    }

    fc1_w = np.asarray(inp["fc1_w"], f32)
    fc2_w = np.asarray(inp["fc2_w"], f32)
    W3e, b3e = _fold_epilogue(np.asarray(inp["fc3_w"], f32),
                              np.asarray(inp["fc3_b"], f32))

    def tiles(vec, p, n):  # [p*n] -> [p, n] with t[j, c] = vec[c*p+j]
        return np.ascontiguousarray(np.asarray(vec, f32).reshape(n, p).T)

    in_maps = []
    for c in range(N_CORES):
        m = dict(common)
        r0 = PREAL * c

        # ---- fc1 shard: W1T[p, kk*1250 + f] = fc1_w[r0+f, feat(p,kk)]
        # feat = ch*94 + 2*kk + par with p = par*64 + ch  (conv3 parity pack)
        A = fc1_w[r0:r0 + PREAL]                       # [1250, 6016]
        A6 = A.reshape(PREAL, NCH, L3H, 2)             # [f, ch, kk, par]
        W1T = A6.transpose(3, 1, 2, 0).reshape(128, L3H, PREAL)
        m["W1T"] = np.ascontiguousarray(
            W1T.reshape(128, L3H * PREAL)).astype(bf16)
        m["fb1"] = tiles(np.asarray(inp["fc1_b"], f32)[r0:r0 + PREAL], 125, 10)
        m["g4"] = tiles(np.asarray(inp["bn4_g"], f32)[r0:r0 + PREAL], 125, 10)
        m["be4"] = tiles(np.asarray(inp["bn4_b"], f32)[r0:r0 + PREAL], 125, 10)

        # ---- fc2 shard (row): W2T[p, j*1250 + f] = fc2_w[r0+f, j*125+p]
        Bc = fc2_w[r0:r0 + PREAL, :]                   # [1250 outs, 10000 in]
        V = Bc.T.reshape(J2, 125, PREAL)               # [j, p, f]
        m["W2T"] = np.ascontiguousarray(
            V.transpose(1, 0, 2).reshape(125, J2 * PREAL)).astype(bf16)
        m["fb2"] = tiles(np.asarray(inp["fc2_b"], f32)[r0:r0 + PREAL], 125, 10)
        m["g5"] = tiles(np.asarray(inp["bn5_g"], f32)[r0:r0 + PREAL], 125, 10)
        m["be5"] = tiles(np.asarray(inp["bn5_b"], f32)[r0:r0 + PREAL], 125, 10)

        # ---- fc3 contraction shard: W3T[p, c2*1200 + f] = W3e[f, r0+c2*125+p]
        C = W3e[:, r0:r0 + PREAL]                      # [1200, 1250]
        V3 = C.T.reshape(10, 125, OUTF)                # [c2, p, f]
        m["W3T"] = np.ascontiguousarray(
            V3.transpose(1, 0, 2).reshape(125, 10 * OUTF)).astype(bf16)
        in_maps.append(m)
    return in_maps, b3e


# ---------------------------------------------------------------- device build

def _build_nc(no_cc=False):
    from concourse import bacc, tile, mybir

    dt = mybir.dt.float32
    bf = mybir.dt.bfloat16
    AF = mybir.ActivationFunctionType
    AL = mybir.AluOpType

    nc = bacc.Bacc("TRN2", target_bir_lowering=False, debug=False,
                   num_devices=N_CORES)

    def din(name, shape, d=dt):
        return nc.dram_tensor(name, list(shape), d, kind="ExternalInput").ap()

    X9 = din("X9", [32, L1 * B], bf)
    w1h = din("w1h", [32, NCH], bf); w1l = din("w1l", [32, NCH], bf)
    cb1 = din("cb1", [NCH])
    w2 = din("w2", [NCH, 3 * NCH]); cb2 = din("cb2", [NCH])
    w3 = din("w3", [NCH, 3 * NCH]); cb3 = din("cb3", [NCH])
    g1 = din("g1", [NCH]); be1 = din("be1", [NCH])
    g2 = din("g2", [NCH]); be2 = din("be2", [NCH])
    g3 = din("g3", [NCH]); be3 = din("be3", [NCH])
    EYE = din("EYE", [B, B])
    W1T = din("W1T", [128, K1 * PREAL], bf)
    fb1 = din("fb1", [125, 10]); g4 = din("g4", [125, 10]); be4 = din("be4", [125, 10])
    W2T = din("W2T", [125, J2 * PREAL], bf)
    fb2 = din("fb2", [125, 10]); g5 = din("g5", [125, 10]); be5 = din("be5", [125, 10])
    W3T = din("W3T", [125, 10 * OUTF], bf)
    out = nc.dram_tensor("out", [B, OUTF], dt, kind="ExternalOutput").ap()

    with tile.TileContext(nc) as tc:
        with (tc.tile_pool(name="const", bufs=1) as cst,
              tc.tile_pool(name="acts", bufs=1) as acts,
              tc.tile_pool(name="wsp", bufs=3) as wsp,
              tc.tile_pool(name="w3p", bufs=1) as w3p,
              tc.tile_pool(name="scr", bufs=2) as scrp,
              tc.tile_pool(name="dram", bufs=1, space="DRAM") as dram):

            # ---- load constants / small tensors
            X9s = cst.tile([32, L1 * B], bf)
            nc.sync.dma_start(X9s[:], X9[:])
            w1hs = cst.tile([32, NCH], bf); nc.sync.dma_start(w1hs[:], w1h[:])
            w1ls = cst.tile([32, NCH], bf); nc.sync.dma_start(w1ls[:], w1l[:])
            w2s = cst.tile([NCH, 3 * NCH], dt); nc.sync.dma_start(w2s[:], w2[:])
            w3s = cst.tile([NCH, 3 * NCH], dt); nc.sync.dma_start(w3s[:], w3[:])
            EYEs = cst.tile([B, B], dt); nc.sync.dma_start(EYEs[:], EYE[:])

            def vec64(ap):
                t = cst.tile([NCH, 1], dt, tag=f"v64_{ap.name}")
                nc.sync.dma_start(t[:], ap[:, None])
                return t
            cb1s, cb2s, cb3s = vec64(cb1), vec64(cb2), vec64(cb3)
            g1s, be1s = vec64(g1), vec64(be1)
            g2s, be2s = vec64(g2), vec64(be2)
            g3s, be3s = vec64(g3), vec64(be3)

            def mat(ap, p, n):
                t = cst.tile([p, n], dt, tag=f"m_{ap.name}")
                nc.sync.dma_start(t[:], ap[:])
                return t
            fb1s, g4s, be4s = mat(fb1, 125, 10), mat(g4, 125, 10), mat(be4, 125, 10)
            fb2s, g5s, be5s = mat(fb2, 125, 10), mat(g5, 125, 10), mat(be5, 125, 10)
            epsb = cst.tile([128, 1], dt, name="epsb")
            nc.vector.memset(epsb[:], EPS)

            # fc3 weights: prefetch early, they're small and needed last
            W3s = w3p.tile([125, 10 * OUTF], bf, name="W3s")
            nc.sync.dma_start(W3s[:], W3T[:])

            # ---- activations
            U1 = acts.tile([NCH, L1 * B], bf)       # relu(conv1+b)
            U2 = acts.tile([NCH, L2 * B], bf)       # relu(conv2'+b2')
            U3 = acts.tile([128, L3H * B], dt)      # relu(conv3'+b3'), parity-packed
            U3b = acts.tile([128, L3H * B], bf)     # bn3 applied, bf16 for fc1
            h1s = acts.tile([B, PREAL], dt)         # fc1 raw out [batch, feat]
            h1 = acts.tile([125, 10 * B], dt)       # fc1 relu out [feat, batch]
            h1b = acts.tile([125, 10 * B], bf)      # bn4 out, bf16
            H1B = acts.tile([125, J2 * B], bf)      # AllGather'd full h1 (10000)
            h2s = acts.tile([B, PREAL], dt)         # fc2 raw out [batch, feat]
            h2r = acts.tile([125, 10 * B], dt)      # relu(fc2+b)
            h2b = acts.tile([125, 10 * B], bf)      # bn5 out
            q3 = acts.tile([B, OUTF], dt)           # fc3 partials

            # stats tiles
            s1sum = cst.tile([NCH, 16], dt); s1sq = cst.tile([NCH, 16], dt)
            s2sum = cst.tile([NCH, 16], dt); s2sq = cst.tile([NCH, 16], dt)
            s3sum = cst.tile([128, 8], dt); s3sq = cst.tile([128, 8], dt)
            h1sum = cst.tile([125, 10], dt); h1sq = cst.tile([125, 10], dt)
            h2sum = cst.tile([125, 10], dt); h2sq = cst.tile([125, 10], dt)

            def bn_vec(pref, p, n):
                return {k: cst.tile([p, n], dt, tag=f"{pref}_{k}",
                                    name=f"{pref}_{k}")
                        for k in ("S", "Q", "m", "mq", "var", "std", "rstd",
                                  "s", "t", "tmp")}

            def bn_from_sums(d, sums, sqs, ntile, count, g_ap, b_ap):
                """Per-partition bn scale/shift from per-tile sums."""
                nc.vector.reduce_sum(d["S"][:], sums[:, 0:ntile], axis=mybir.AxisListType.X)
                nc.vector.reduce_sum(d["Q"][:], sqs[:, 0:ntile], axis=mybir.AxisListType.X)
                inv = 1.0 / count
                nc.vector.tensor_scalar_mul(d["m"][:], d["S"][:], inv)
                nc.vector.tensor_scalar_mul(d["mq"][:], d["Q"][:], inv)
                nc.vector.tensor_tensor(d["tmp"][:], d["m"][:], d["m"][:], op=AL.mult)
                nc.vector.tensor_tensor(d["var"][:], d["mq"][:], d["tmp"][:], op=AL.subtract)
                nc.scalar.activation(d["std"][:], d["var"][:], AF.Sqrt,
                                     bias=epsb[0:d["var"].shape[0], :])
                nc.vector.reciprocal(d["rstd"][:], d["std"][:])
                nc.vector.tensor_tensor(d["s"][:], g_ap, d["rstd"][:], op=AL.mult)
                nc.vector.tensor_tensor(d["tmp"][:], d["m"][:], d["s"][:], op=AL.mult)
                nc.vector.tensor_tensor(d["t"][:], b_ap, d["tmp"][:], op=AL.subtract)

            # =========================================================
            # conv1: U1 = relu(w1.T @ X9 + cb1)
            ps_cm = tc.tile_pool(name="pscv", bufs=2, space="PSUM")
            ps = ps_cm.__enter__()
            n1 = L1 * B  # 6272
            t1sizes = [512] * 12 + [128]
            for t in range(13):
                sz = t1sizes[t]
                pt = ps.tile([NCH, 512], dt, tag="cps")
                nc.tensor.matmul(pt[:, 0:sz], w1hs[:],
                                 X9s[:, 512 * t:512 * t + sz],
                                 start=True, stop=False)
                nc.tensor.matmul(pt[:, 0:sz], w1ls[:],
                                 X9s[:, 512 * t:512 * t + sz],
                                 start=False, stop=True)
                nc.scalar.activation(U1[:, 512 * t:512 * t + sz], pt[:, 0:sz],
                                     AF.Relu, bias=cb1s[:], accum_out=s1sum[:, t:t + 1])
                sc = scrp.tile([128, 512], dt, tag="scr", name="sc")
                nc.scalar.activation(sc[0:NCH, 0:sz], U1[:, 512 * t:512 * t + sz],
                                     AF.Square, accum_out=s1sq[:, t:t + 1])

            bn1 = bn_vec("bn1", NCH, 1)
            bn_from_sums(bn1, s1sum, s1sq, 13, float(n1), g1s[:], be1s[:])

            # fold bn1 into conv2 weights: w2f = w2 * s1 (per in-channel),
            # b2f = cb2 + sum_k w2[k].T @ t1
            w2f = cst.tile([NCH, 3 * NCH], dt)
            nc.vector.tensor_scalar_mul(w2f[:], w2s[:], bn1["s"][:])
            w2fh = cst.tile([NCH, 3 * NCH], bf)
            nc.vector.tensor_copy(w2fh[:], w2f[:])
            w2fh32 = cst.tile([NCH, 3 * NCH], dt)
            nc.vector.tensor_copy(w2fh32[:], w2fh[:])
            w2fl = cst.tile([NCH, 3 * NCH], bf)
            nc.vector.tensor_tensor(w2fl[:], w2f[:], w2fh32[:], op=AL.subtract)
            pb = ps.tile([NCH, 1], dt, tag="cpsb")
            for k in range(3):
                nc.tensor.matmul(pb[:], w2s[:, 64 * k:64 * k + 64], bn1["t"][:],
                                 start=(k == 0), stop=(k == 2))
            b2f = cst.tile([NCH, 1], dt)
            nc.vector.tensor_tensor(b2f[:], pb[:], cb2s[:], op=AL.add)

            # =========================================================
            # conv2: U2 = relu(w2f.T conv U1 + b2f)
            for t in range(12):
                pt = ps.tile([NCH, 512], dt, tag="cps")
                for hk in range(6):
                    h, k = hk // 3, hk % 3
                    wsrc = w2fh if h == 0 else w2fl
                    nc.tensor.matmul(pt[:], wsrc[:, 64 * k:64 * k + 64],
                                     U1[:, (8 * t + k) * B:(8 * t + k) * B + 512],
                                     start=(hk == 0), stop=(hk == 5))
                nc.scalar.activation(U2[:, 512 * t:512 * t + 512], pt[:],
                                     AF.Relu, bias=b2f[:], accum_out=s2sum[:, t:t + 1])
                sc = scrp.tile([128, 512], dt, tag="scr", name="sc")
                nc.scalar.activation(sc[0:NCH, :], U2[:, 512 * t:512 * t + 512],
                                     AF.Square, accum_out=s2sq[:, t:t + 1])

            bn2 = bn_vec("bn2", NCH, 1)
            bn_from_sums(bn2, s2sum, s2sq, 12, float(L2 * B), g2s[:], be2s[:])

            w3f = cst.tile([NCH, 3 * NCH], dt)
            nc.vector.tensor_scalar_mul(w3f[:], w3s[:], bn2["s"][:])
            w3fh = cst.tile([NCH, 3 * NCH], bf)
            nc.vector.tensor_copy(w3fh[:], w3f[:])
            w3fh32 = cst.tile([NCH, 3 * NCH], dt)
            nc.vector.tensor_copy(w3fh32[:], w3fh[:])
            w3fl = cst.tile([NCH, 3 * NCH], bf)
            nc.vector.tensor_tensor(w3fl[:], w3f[:], w3fh32[:], op=AL.subtract)
            pb3 = ps.tile([NCH, 1], dt, tag="cpsb")
            for k in range(3):
                nc.tensor.matmul(pb3[:], w3s[:, 64 * k:64 * k + 64], bn2["t"][:],
                                 start=(k == 0), stop=(k == 2))
            b3f = cst.tile([NCH, 1], dt)
            nc.vector.tensor_tensor(b3f[:], pb3[:], cb3s[:], op=AL.add)
            b3d = cst.tile([128, 1], dt)
            nc.vector.tensor_copy(b3d[0:NCH, :], b3f[:])
            nc.vector.tensor_copy(b3d[NCH:128, :], b3f[:])

            # =========================================================
            # conv3 (parity-packed): U3[par*64+c, l2*64+b] = relu(conv3')
            U2v = U2[:].rearrange("p (l two b) -> p two l b", two=2, b=B)
            t3l2 = [8, 8, 8, 8, 8, 7]   # 47 l2 positions
            for t in range(6):
                lw = t3l2[t]
                pt = ps.tile([128, 512], dt, tag="cps3")
                for par in range(2):
                    for hk in range(6):
                        h, k = hk // 3, hk % 3
                        wsrc = w3fh if h == 0 else w3fl
                        pk = par + k
                        rhs = U2v[:, pk % 2, 8 * t + pk // 2: 8 * t + pk // 2 + lw, :]
                        nc.tensor.matmul(pt[64 * par:64 * par + 64, 0:64 * lw],
                                         wsrc[:, 64 * k:64 * k + 64], rhs,
                                         start=(hk == 0), stop=(hk == 5),
                                         tile_position=(0, 64 * par))
                nc.scalar.activation(U3[:, 512 * t:512 * t + 64 * lw], pt[:, 0:64 * lw],
                                     AF.Relu, bias=b3d[:], accum_out=s3sum[:, t:t + 1])
                sc = scrp.tile([128, 512], dt, tag="scr")
                nc.scalar.activation(sc[:, 0:64 * lw], U3[:, 512 * t:512 * t + 64 * lw],
                                     AF.Square, accum_out=s3sq[:, t:t + 1])

            # bn3: combine parity halves, then broadcast back to 128 partitions
            S3 = cst.tile([128, 1], dt); Q3 = cst.tile([128, 1], dt)
            nc.vector.reduce_sum(S3[:], s3sum[:, 0:6], axis=mybir.AxisListType.X)
            nc.vector.reduce_sum(Q3[:], s3sq[:, 0:6], axis=mybir.AxisListType.X)
            cS = cst.tile([NCH, 1], dt); cQ = cst.tile([NCH, 1], dt)
            nc.vector.tensor_copy(cS[:], S3[NCH:128, :])
            nc.vector.tensor_copy(cQ[:], Q3[NCH:128, :])
            St = cst.tile([NCH, 1], dt); Qt = cst.tile([NCH, 1], dt)
            nc.vector.tensor_tensor(St[:], S3[0:NCH, :], cS[:], op=AL.add)
            nc.vector.tensor_tensor(Qt[:], Q3[0:NCH, :], cQ[:], op=AL.add)

            bn3 = bn_vec("bn3", NCH, 1)
            inv3 = 1.0 / float(H)
            nc.vector.tensor_scalar_mul(bn3["m"][:], St[:], inv3)
            nc.vector.tensor_scalar_mul(bn3["mq"][:], Qt[:], inv3)
            nc.vector.tensor_tensor(bn3["tmp"][:], bn3["m"][:], bn3["m"][:], op=AL.mult)
            nc.vector.tensor_tensor(bn3["var"][:], bn3["mq"][:], bn3["tmp"][:], op=AL.subtract)
            nc.scalar.activation(bn3["std"][:], bn3["var"][:], AF.Sqrt, bias=epsb[0:NCH, :])
            nc.vector.reciprocal(bn3["rstd"][:], bn3["std"][:])
            nc.vector.tensor_tensor(bn3["s"][:], g3s[:], bn3["rstd"][:], op=AL.mult)
            nc.vector.tensor_tensor(bn3["tmp"][:], bn3["m"][:], bn3["s"][:], op=AL.mult)
            nc.vector.tensor_tensor(bn3["t"][:], be3s[:], bn3["tmp"][:], op=AL.subtract)
            s3b = cst.tile([128, 1], dt); t3b = cst.tile([128, 1], dt)
            nc.vector.tensor_copy(s3b[0:NCH, :], bn3["s"][:])
            nc.vector.tensor_copy(s3b[NCH:128, :], bn3["s"][:])
            nc.vector.tensor_copy(t3b[0:NCH, :], bn3["t"][:])
            nc.vector.tensor_copy(t3b[NCH:128, :], bn3["t"][:])
            nc.vector.tensor_scalar(U3b[:], U3[:], s3b[:], t3b[:],
                                    op0=AL.mult, op1=AL.add)
            ps_cm.__exit__(None, None, None)

            psA_cm = tc.tile_pool(name="psacc", bufs=1, space="PSUM")
            psA = psA_cm.__enter__()
            psT_cm = tc.tile_pool(name="pstp", bufs=2, space="PSUM")
            psT = psT_cm.__enter__()

            # =========================================================
            # fc1 (weights moving): psum[b, f] += U3b_kk.T @ W1T_kk
            acc = [psA.tile([128, 512], dt, tag=f"acc{i}", name=f"acc{i}")
                   for i in range(3)]
            kk = 0
            for gn in G1:
                Wg = wsp.tile([128, 8 * PREAL], bf, tag="wst", name="Wg")
                nc.sync.dma_start(Wg[:, 0:gn * PREAL],
                                  W1T[:, kk * PREAL:(kk + gn) * PREAL])
                for kl in range(gn):
                    for i, (f0, fw) in enumerate(FT):
                        nc.tensor.matmul(acc[i][0:B, 0:fw],
                                         U3b[:, B * kk:B * kk + B],
                                         Wg[:, kl * PREAL + f0:kl * PREAL + f0 + fw],
                                         start=(kk == 0), stop=(kk == K1 - 1))
                    kk += 1
            for i, (f0, fw) in enumerate(FT):
                nc.vector.tensor_copy(h1s[:, f0:f0 + fw], acc[i][0:B, 0:fw])

            # transpose each 125-chunk back to [feat, batch]; relu+bias+stats
            def bn_feat(pref, sums, sqs, g_ap, b_ap):
                d = bn_vec(pref, 125, 10)
                inv = 1.0 / float(B)
                nc.vector.tensor_scalar_mul(d["m"][:], sums[:], inv)
                nc.vector.tensor_scalar_mul(d["mq"][:], sqs[:], inv)
                nc.vector.tensor_tensor(d["tmp"][:], d["m"][:], d["m"][:], op=AL.mult)
                nc.vector.tensor_tensor(d["var"][:], d["mq"][:], d["tmp"][:], op=AL.subtract)
                nc.scalar.activation(d["std"][:], d["var"][:], AF.Sqrt,
                                     bias=epsb[0:125, :])
                nc.vector.reciprocal(d["rstd"][:], d["std"][:])
                nc.vector.tensor_tensor(d["s"][:], g_ap, d["rstd"][:], op=AL.mult)
                nc.vector.tensor_tensor(d["tmp"][:], d["m"][:], d["s"][:], op=AL.mult)
                nc.vector.tensor_tensor(d["t"][:], b_ap, d["tmp"][:], op=AL.subtract)
                return d

            for c2 in range(10):
                tp = psT.tile([125, 64], dt, tag="tp", name="tp")
                nc.tensor.transpose(tp[:], h1s[:, 125 * c2:125 * c2 + 125], EYEs[:])
                nc.scalar.activation(h1[:, B * c2:B * c2 + B], tp[:],
                                     AF.Relu, bias=fb1s[:, c2:c2 + 1],
                                     accum_out=h1sum[:, c2:c2 + 1])
                sc = scrp.tile([128, 512], dt, tag="scr", name="sc")
                nc.scalar.activation(sc[0:125, 0:B], h1[:, B * c2:B * c2 + B],
                                     AF.Square, accum_out=h1sq[:, c2:c2 + 1])

            bn4 = bn_feat("bn4", h1sum, h1sq, g4s[:], be4s[:])
            for c2 in range(10):
                nc.vector.tensor_scalar(h1b[:, B * c2:B * c2 + B],
                                        h1[:, B * c2:B * c2 + B],
                                        bn4["s"][:, c2:c2 + 1], bn4["t"][:, c2:c2 + 1],
                                        op0=AL.mult, op1=AL.add)

            # =========================================================
            # AllGather the local bn4 output (125 x 640 bf16 = 160 KB)
            agin = dram.tile([125, 10 * B], bf)
            agout = dram.tile([N_CORES * 125, 10 * B], bf)
            nc.scalar.dma_start(agin[:], h1b[:])
            if no_cc:
                for r in range(N_CORES):
                    nc.scalar.dma_start(agout[125 * r:125 * (r + 1), :], agin[:])
            else:
                nc.gpsimd.collective_compute(
                    "AllGather", mybir.AluOpType.bypass,
                    replica_groups=[list(range(N_CORES))],
                    ins=[agin[:]], outs=[agout[:]])
            nc.scalar.dma_start(
                H1B[:].rearrange("p (r n) -> p r n", r=N_CORES),
                agout[:].rearrange("(r p) n -> p r n", p=125))

            # =========================================================
            # fc2 (weights moving, row-sharded): full 10000 contraction
            acc2 = [psA.tile([128, 512], dt, tag=f"acc{i}", name=f"acc2_{i}")
                    for i in range(3)]
            for g in range(G2N):
                Wg = wsp.tile([128, 8 * PREAL], bf, tag="wst", name="Wg2")
                nc.sync.dma_start(Wg[0:125, :],
                                  W2T[:, g * 8 * PREAL:(g + 1) * 8 * PREAL])
                for jl in range(8):
                    j = 8 * g + jl
                    for i, (f0, fw) in enumerate(FT):
                        nc.tensor.matmul(acc2[i][0:B, 0:fw],
                                         H1B[:, B * j:B * j + B],
                                         Wg[0:125, jl * PREAL + f0:jl * PREAL + f0 + fw],
                                         start=(j == 0), stop=(j == J2 - 1))
            for i, (f0, fw) in enumerate(FT):
                nc.vector.tensor_copy(h2s[:, f0:f0 + fw], acc2[i][0:B, 0:fw])

            for c2 in range(10):
                tp = psT.tile([125, 64], dt, tag="tp", name="tp2")
                nc.tensor.transpose(tp[:], h2s[:, 125 * c2:125 * c2 + 125], EYEs[:])
                nc.scalar.activation(h2r[:, B * c2:B * c2 + B], tp[:],
                                     AF.Relu, bias=fb2s[:, c2:c2 + 1],
                                     accum_out=h2sum[:, c2:c2 + 1])
                sc = scrp.tile([128, 512], dt, tag="scr", name="sc")
                nc.scalar.activation(sc[0:125, 0:B], h2r[:, B * c2:B * c2 + B],
                                     AF.Square, accum_out=h2sq[:, c2:c2 + 1])

            bn5 = bn_feat("bn5", h2sum, h2sq, g5s[:], be5s[:])
            for c2 in range(10):
                nc.vector.tensor_scalar(h2b[:, B * c2:B * c2 + B],
                                        h2r[:, B * c2:B * c2 + B],
                                        bn5["s"][:, c2:c2 + 1], bn5["t"][:, c2:c2 + 1],
                                        op0=AL.mult, op1=AL.add)

            # =========================================================
            # fc3 (weights moving, contraction-sharded; epilogue in weights)
            acc3 = [psA.tile([128, 512], dt, tag=f"acc{i}", name=f"acc3_{i}")
                    for i in range(3)]
            for c2 in range(10):
                for i, (f0, fw) in enumerate(FT3):
                    nc.tensor.matmul(acc3[i][0:B, 0:fw],
                                     h2b[:, B * c2:B * c2 + B],
                                     W3s[:, c2 * OUTF + f0:c2 * OUTF + f0 + fw],
                                     start=(c2 == 0), stop=(c2 == 9))
            for i, (f0, fw) in enumerate(FT3):
                nc.vector.tensor_copy(q3[:, f0:f0 + fw], acc3[i][0:B, 0:fw])

            nc.scalar.dma_start(out[:], q3[:])
            psT_cm.__exit__(None, None, None)
            psA_cm.__exit__(None, None, None)

    nc.compile()
    return nc


# ---------------------------------------------------------------- entry point

def _run_sim(nc, in_maps):
    from concourse.bass_interp import MultiCoreSim

    sim = MultiCoreSim(nc, num_cores=N_CORES, trace=False,
                       require_finite=False, require_nnan=False)
    for i, (cid, core) in enumerate(sim.cores.items()):
        for name, arr in in_maps[i].items():
            core.tensor(name)[:] = arr
    sim.simulate(check_with_hw=False)
    return [np.array(sim.cores[c].tensor("out")) for c in range(N_CORES)]


def _finish(outs, b3e):
    acc = np.zeros((B, OUTF), np.float64)
    for o in outs:
        acc += np.asarray(o, np.float64)
    acc += b3e.astype(np.float64)[None, :]
    return np.ascontiguousarray(acc).reshape(B, 12, 100).astype(np.float32)


def kernel(**inputs):
    from concourse import bass_utils

    if "nc" not in _CACHE:
        _CACHE["nc"] = _build_nc()
    nc = _CACHE["nc"]

    in_maps, b3e = _prep_in_maps(inputs)
    outs = None
    for attempt in range(2):
        try:
            res = bass_utils.run_bass_kernel_spmd(
                nc, in_maps, core_ids=list(range(N_CORES)))
            outs = [res.results[i]["out"] for i in range(N_CORES)]
            print(f"[kernel] hw run ok (attempt {attempt})", file=sys.stderr)
            break
        except Exception as e:
            # device may be wedged from a prior run; one retry usually
            # recovers it. After that, fall back to the simulator.
            print(f"[kernel] hw attempt {attempt} failed: {type(e).__name__}",
                  file=sys.stderr)
            continue
    if outs is None:
        print("[kernel] falling back to simulator", file=sys.stderr)
        outs = _run_sim(nc, in_maps)
    return _finish(outs, b3e)


# revision 9
# speedup vs baseline: 2.0929x; 2.0929x over previous
"""Trainium2 Bass kernel for nn_Conv1dMapper (3x conv1d+bn -> 3x fc+bn -> interp epilogue).

Self-contained: accepts FULL inputs, shards across 8 NeuronCores internally,
returns the FULL [64, 12, 100] output.

Sharding strategy (v3, weights-moving):
  - conv stage (tiny) replicated on all cores in bf16; bn1/bn2 folded into the
    next conv's weights at runtime; conv3 packs even/odd output positions into
    128 partitions via PE column-tiling so fc1 gets K=128 contraction chunks.
  - ALL fc matmuls are "weights-moving": the activations (batch=64 wide) are
    the stationary operand and the weights stream through the PE array with a
    512-wide free dim.  This turns each fc layer into ~N_chunks*3 large
    matmuls instead of thousands of N=64 ones.
  - fc1 row-sharded (1250 rows/core).  Output lands as [batch, feat] in PSUM;
    PE-transpose (identity matmul) flips each 125-chunk back to [feat, batch],
    where bias+relu+bn4 run exactly like a weights-stationary kernel.
  - the bf16 bn4 output (160 KB) is AllGather'd so every core holds the full
    10000-dim h1; fc2 is then row-sharded too (1250 outs/core, full
    contraction) - no ReduceScatter of fp32 partials needed.
  - fc3 contraction-sharded with the output epilogue folded into its weights;
    fp32 partials [64, 1200] are returned per-core and summed on the host.
"""

import sys

sys.path.insert(0, "/opt/trn_rl_repo")

import numpy as np

N_CORES = 8
B = 64            # batch
L1, L2, L3 = 98, 96, 94
NCH = 64          # conv channels
H = 6016          # fc1 in features = 64*94
L3H = 47          # = L3 // 2
HID = 10000
PREAL = 1250      # fc1/fc2 output rows per core
EPS = 1e-5
OUTF = 1200

K1 = 47           # fc1 contraction chunks (128 each)
J2 = 80           # fc2 contraction chunks (125 each)
FT = [(0, 512), (512, 512), (1024, 226)]    # f-tiles over 1250 local outs
FT3 = [(0, 512), (512, 512), (1024, 176)]   # f-tiles over 1200 outs
G1 = [8, 8, 8, 8, 8, 7]                     # fc1 kk-chunk DMA groups
G2N = 10                                    # fc2 j-chunk DMA groups (8 each)

_CACHE = {}


# ---------------------------------------------------------------- host prep

def _fold_epilogue(fc3_w, fc3_b):
    """Fold reshape->zero/one channels->interpolation into fc3's weights."""
    L = 100
    CD = L // 3
    CPS = np.array([1, CD, 2 * CD, 3 * CD])
    REG = np.array([p for p in range(L) if p not in set(CPS.tolist())][1:])
    J = REG // CD
    Lp = CPS[J]
    Rp = CPS[J + 1]
    ALPHA = ((REG - Lp) / CD).astype(np.float32)
    CH = np.array([0, 2, 8, 10, 3, 11])
    ZERO_CH = np.array([1, 4, 6, 7, 9])

    W3e = fc3_w.astype(np.float32).copy()
    b3e = fc3_b.astype(np.float32).copy()
    idx0 = (ZERO_CH[:, None] * L + np.arange(L)[None, :]).ravel()
    W3e[idx0] = 0.0
    b3e[idx0] = 0.0
    idx1 = 5 * L + np.arange(L)
    W3e[idx1] = 0.0
    b3e[idx1] = 1.0
    rows_t = (CH[:, None] * L + REG[None, :]).ravel()
    rows_l = (CH[:, None] * L + Lp[None, :]).ravel()
    rows_r = (CH[:, None] * L + Rp[None, :]).ravel()
    a = np.broadcast_to(ALPHA[None, :], (len(CH), len(REG))).ravel()[:, None]
    W3e[rows_t] = a * fc3_w[rows_l] + (1.0 - a) * fc3_w[rows_r]
    b3e[rows_t] = (a[:, 0] * fc3_b[rows_l] + (1.0 - a[:, 0]) * fc3_b[rows_r])
    return W3e, b3e


def _prep_in_maps(inp):
    from concourse import mybir

    f32 = np.float32
    bf16 = mybir.dt.np(mybir.dt.bfloat16)
    x = np.asarray(inp["x"], f32)

    # conv1 im2col: X9[k*3+i, l*64+b] = x[b, i, l+k]
    x_t = np.ascontiguousarray(x.transpose(1, 2, 0))      # [3, 100, 64]
    X9 = np.stack([x_t[:, k:k + L1, :] for k in range(3)], 0)  # [k, i, l, b]
    X9 = X9.reshape(9, L1 * B)
    X9 = np.ascontiguousarray(
        np.concatenate([X9, np.zeros((23, L1 * B), f32)], 0)).astype(bf16)

    w1 = np.asarray(inp["conv1_w"], f32).transpose(2, 1, 0).reshape(9, NCH)
    w1 = np.ascontiguousarray(
        np.concatenate([w1, np.zeros((23, NCH), f32)], 0))
    w1h = w1.astype(bf16)
    w1l = (w1 - w1h.astype(f32)).astype(bf16)
    w2 = np.ascontiguousarray(
        np.asarray(inp["conv2_w"], f32).transpose(1, 2, 0).reshape(NCH, 3 * NCH))
    w3 = np.ascontiguousarray(
        np.asarray(inp["conv3_w"], f32).transpose(1, 2, 0).reshape(NCH, 3 * NCH))

    common = {
        "X9": X9, "w1h": w1h, "w1l": w1l, "w2": w2, "w3": w3,
        "cb1": np.asarray(inp["conv1_b"], f32),
        "cb2": np.asarray(inp["conv2_b"], f32),
        "cb3": np.asarray(inp["conv3_b"], f32),
        "g1": np.asarray(inp["bn1_g"], f32), "be1": np.asarray(inp["bn1_b"], f32),
        "g2": np.asarray(inp["bn2_g"], f32), "be2": np.asarray(inp["bn2_b"], f32),
        "g3": np.asarray(inp["bn3_g"], f32), "be3": np.asarray(inp["bn3_b"], f32),
        "EYE": np.eye(B, dtype=f32),
    }

    fc1_w = np.asarray(inp["fc1_w"], f32)
    fc2_w = np.asarray(inp["fc2_w"], f32)
    W3e, b3e = _fold_epilogue(np.asarray(inp["fc3_w"], f32),
                              np.asarray(inp["fc3_b"], f32))

    def tiles(vec, p, n):  # [p*n] -> [p, n] with t[j, c] = vec[c*p+j]
        return np.ascontiguousarray(np.asarray(vec, f32).reshape(n, p).T)

    in_maps = []
    for c in range(N_CORES):
        m = dict(common)
        r0 = PREAL * c

        # ---- fc1 shard: W1T[p, kk*1250 + f] = fc1_w[r0+f, feat(p,kk)]
        # feat = ch*94 + 2*kk + par with p = par*64 + ch  (conv3 parity pack)
        A = fc1_w[r0:r0 + PREAL]                       # [1250, 6016]
        A6 = A.reshape(PREAL, NCH, L3H, 2)             # [f, ch, kk, par]
        W1T = A6.transpose(3, 1, 2, 0).reshape(128, L3H, PREAL)
        m["W1T"] = np.ascontiguousarray(
            W1T.reshape(128, L3H * PREAL)).astype(bf16)
        m["fb1"] = tiles(np.asarray(inp["fc1_b"], f32)[r0:r0 + PREAL], 125, 10)
        m["g4"] = tiles(np.asarray(inp["bn4_g"], f32)[r0:r0 + PREAL], 125, 10)
        m["be4"] = tiles(np.asarray(inp["bn4_b"], f32)[r0:r0 + PREAL], 125, 10)

        # ---- fc2 shard (row): W2T[p, j*1250 + f] = fc2_w[r0+f, j*125+p]
        Bc = fc2_w[r0:r0 + PREAL, :]                   # [1250 outs, 10000 in]
        V = Bc.T.reshape(J2, 125, PREAL)               # [j, p, f]
        m["W2T"] = np.ascontiguousarray(
            V.transpose(1, 0, 2).reshape(125, J2 * PREAL)).astype(bf16)
        m["fb2"] = tiles(np.asarray(inp["fc2_b"], f32)[r0:r0 + PREAL], 125, 10)
        m["g5"] = tiles(np.asarray(inp["bn5_g"], f32)[r0:r0 + PREAL], 125, 10)
        m["be5"] = tiles(np.asarray(inp["bn5_b"], f32)[r0:r0 + PREAL], 125, 10)

        # ---- fc3 contraction shard: W3T[p, c2*1200 + f] = W3e[f, r0+c2*125+p]
        C = W3e[:, r0:r0 + PREAL]                      # [1200, 1250]
        V3 = C.T.reshape(10, 125, OUTF)                # [c2, p, f]
        m["W3T"] = np.ascontiguousarray(
            V3.transpose(1, 0, 2).reshape(125, 10 * OUTF)).astype(bf16)
        in_maps.append(m)
    return in_maps, b3e


# ---------------------------------------------------------------- device build

def _build_nc(no_cc=False):
    from concourse import bacc, tile, mybir

    dt = mybir.dt.float32
    bf = mybir.dt.bfloat16
    AF = mybir.ActivationFunctionType
    AL = mybir.AluOpType

    nc = bacc.Bacc("TRN2", target_bir_lowering=False, debug=False,
                   num_devices=N_CORES)

    def din(name, shape, d=dt):
        return nc.dram_tensor(name, list(shape), d, kind="ExternalInput").ap()

    X9 = din("X9", [32, L1 * B], bf)
    w1h = din("w1h", [32, NCH], bf); w1l = din("w1l", [32, NCH], bf)
    cb1 = din("cb1", [NCH])
    w2 = din("w2", [NCH, 3 * NCH]); cb2 = din("cb2", [NCH])
    w3 = din("w3", [NCH, 3 * NCH]); cb3 = din("cb3", [NCH])
    g1 = din("g1", [NCH]); be1 = din("be1", [NCH])
    g2 = din("g2", [NCH]); be2 = din("be2", [NCH])
    g3 = din("g3", [NCH]); be3 = din("be3", [NCH])
    EYE = din("EYE", [B, B])
    W1T = din("W1T", [128, K1 * PREAL], bf)
    fb1 = din("fb1", [125, 10]); g4 = din("g4", [125, 10]); be4 = din("be4", [125, 10])
    W2T = din("W2T", [125, J2 * PREAL], bf)
    fb2 = din("fb2", [125, 10]); g5 = din("g5", [125, 10]); be5 = din("be5", [125, 10])
    W3T = din("W3T", [125, 10 * OUTF], bf)
    out = nc.dram_tensor("out", [B, OUTF], dt, kind="ExternalOutput").ap()

    with tile.TileContext(nc) as tc:
        with (tc.tile_pool(name="const", bufs=1) as cst,
              tc.tile_pool(name="acts", bufs=1) as acts,
              tc.tile_pool(name="wsp", bufs=3) as wsp,
              tc.tile_pool(name="w3p", bufs=1) as w3p,
              tc.tile_pool(name="scr", bufs=2) as scrp,
              tc.tile_pool(name="dram", bufs=1, space="DRAM") as dram):

            # ---- load constants / small tensors
            X9s = cst.tile([32, L1 * B], bf)
            nc.sync.dma_start(X9s[:], X9[:])
            w1hs = cst.tile([32, NCH], bf); nc.sync.dma_start(w1hs[:], w1h[:])
            w1ls = cst.tile([32, NCH], bf); nc.sync.dma_start(w1ls[:], w1l[:])
            w2s = cst.tile([NCH, 3 * NCH], dt); nc.sync.dma_start(w2s[:], w2[:])
            w3s = cst.tile([NCH, 3 * NCH], dt); nc.sync.dma_start(w3s[:], w3[:])
            EYEs = cst.tile([B, B], dt); nc.sync.dma_start(EYEs[:], EYE[:])

            def vec64(ap):
                t = cst.tile([NCH, 1], dt, tag=f"v64_{ap.name}")
                nc.sync.dma_start(t[:], ap[:, None])
                return t
            cb1s, cb2s, cb3s = vec64(cb1), vec64(cb2), vec64(cb3)
            g1s, be1s = vec64(g1), vec64(be1)
            g2s, be2s = vec64(g2), vec64(be2)
            g3s, be3s = vec64(g3), vec64(be3)

            def mat(ap, p, n):
                t = cst.tile([p, n], dt, tag=f"m_{ap.name}")
                nc.sync.dma_start(t[:], ap[:])
                return t
            fb1s, g4s, be4s = mat(fb1, 125, 10), mat(g4, 125, 10), mat(be4, 125, 10)
            fb2s, g5s, be5s = mat(fb2, 125, 10), mat(g5, 125, 10), mat(be5, 125, 10)
            epsb = cst.tile([128, 1], dt, name="epsb")
            nc.vector.memset(epsb[:], EPS)

            # fc3 weights: prefetch early, they're small and needed last
            W3s = w3p.tile([125, 10 * OUTF], bf, name="W3s")
            nc.sync.dma_start(W3s[:], W3T[:])

            # ---- activations
            U1 = acts.tile([NCH, L1 * B], bf)       # relu(conv1+b)
            U2 = acts.tile([NCH, L2 * B], bf)       # relu(conv2'+b2')
            U3 = acts.tile([128, L3H * B], dt)      # relu(conv3'+b3'), parity-packed
            U3b = acts.tile([128, L3H * B], bf)     # bn3 applied, bf16 for fc1
            h1s = acts.tile([B, PREAL], dt)         # fc1 raw out [batch, feat]
            h1 = acts.tile([125, 10 * B], dt)       # fc1 relu out [feat, batch]
            h1b = acts.tile([125, 10 * B], bf)      # bn4 out, bf16
            H1B = acts.tile([125, J2 * B], bf)      # AllGather'd full h1 (10000)
            h2s = acts.tile([B, PREAL], dt)         # fc2 raw out [batch, feat]
            h2r = acts.tile([125, 10 * B], dt)      # relu(fc2+b)
            h2b = acts.tile([125, 10 * B], bf)      # bn5 out
            q3 = acts.tile([B, OUTF], dt)           # fc3 partials

            # stats tiles
            s1sum = cst.tile([NCH, 16], dt); s1sq = cst.tile([NCH, 16], dt)
            s2sum = cst.tile([NCH, 16], dt); s2sq = cst.tile([NCH, 16], dt)
            s3sum = cst.tile([128, 8], dt); s3sq = cst.tile([128, 8], dt)
            h1sum = cst.tile([125, 10], dt); h1sq = cst.tile([125, 10], dt)
            h2sum = cst.tile([125, 10], dt); h2sq = cst.tile([125, 10], dt)

            def bn_vec(pref, p, n):
                return {k: cst.tile([p, n], dt, tag=f"{pref}_{k}",
                                    name=f"{pref}_{k}")
                        for k in ("S", "Q", "m", "mq", "var", "std", "rstd",
                                  "s", "t", "tmp")}

            def bn_from_sums(d, sums, sqs, ntile, count, g_ap, b_ap):
                """Per-partition bn scale/shift from per-tile sums."""
                nc.vector.reduce_sum(d["S"][:], sums[:, 0:ntile], axis=mybir.AxisListType.X)
                nc.vector.reduce_sum(d["Q"][:], sqs[:, 0:ntile], axis=mybir.AxisListType.X)
                inv = 1.0 / count
                nc.vector.tensor_scalar_mul(d["m"][:], d["S"][:], inv)
                nc.vector.tensor_scalar_mul(d["mq"][:], d["Q"][:], inv)
                nc.vector.tensor_tensor(d["tmp"][:], d["m"][:], d["m"][:], op=AL.mult)
                nc.vector.tensor_tensor(d["var"][:], d["mq"][:], d["tmp"][:], op=AL.subtract)
                nc.scalar.activation(d["std"][:], d["var"][:], AF.Sqrt,
                                     bias=epsb[0:d["var"].shape[0], :])
                nc.vector.reciprocal(d["rstd"][:], d["std"][:])
                nc.vector.tensor_tensor(d["s"][:], g_ap, d["rstd"][:], op=AL.mult)
                nc.vector.tensor_tensor(d["tmp"][:], d["m"][:], d["s"][:], op=AL.mult)
                nc.vector.tensor_tensor(d["t"][:], b_ap, d["tmp"][:], op=AL.subtract)

            # =========================================================
            # conv1: U1 = relu(w1.T @ X9 + cb1)
            ps_cm = tc.tile_pool(name="pscv", bufs=2, space="PSUM")
            ps = ps_cm.__enter__()
            n1 = L1 * B  # 6272
            t1sizes = [512] * 12 + [128]
            for t in range(13):
                sz = t1sizes[t]
                pt = ps.tile([NCH, 512], dt, tag="cps")
                nc.tensor.matmul(pt[:, 0:sz], w1hs[:],
                                 X9s[:, 512 * t:512 * t + sz],
                                 start=True, stop=False)
                nc.tensor.matmul(pt[:, 0:sz], w1ls[:],
                                 X9s[:, 512 * t:512 * t + sz],
                                 start=False, stop=True)
                nc.scalar.activation(U1[:, 512 * t:512 * t + sz], pt[:, 0:sz],
                                     AF.Relu, bias=cb1s[:], accum_out=s1sum[:, t:t + 1])
                sc = scrp.tile([128, 512], dt, tag="scr", name="sc")
                nc.scalar.activation(sc[0:NCH, 0:sz], U1[:, 512 * t:512 * t + sz],
                                     AF.Square, accum_out=s1sq[:, t:t + 1])

            bn1 = bn_vec("bn1", NCH, 1)
            bn_from_sums(bn1, s1sum, s1sq, 13, float(n1), g1s[:], be1s[:])

            # fold bn1 into conv2 weights: w2f = w2 * s1 (per in-channel),
            # b2f = cb2 + sum_k w2[k].T @ t1
            w2f = cst.tile([NCH, 3 * NCH], dt)
            nc.vector.tensor_scalar_mul(w2f[:], w2s[:], bn1["s"][:])
            w2fh = cst.tile([NCH, 3 * NCH], bf)
            nc.vector.tensor_copy(w2fh[:], w2f[:])
            w2fh32 = cst.tile([NCH, 3 * NCH], dt)
            nc.vector.tensor_copy(w2fh32[:], w2fh[:])
            w2fl = cst.tile([NCH, 3 * NCH], bf)
            nc.vector.tensor_tensor(w2fl[:], w2f[:], w2fh32[:], op=AL.subtract)
            pb = ps.tile([NCH, 1], dt, tag="cpsb")
            for k in range(3):
                nc.tensor.matmul(pb[:], w2s[:, 64 * k:64 * k + 64], bn1["t"][:],
                                 start=(k == 0), stop=(k == 2))
            b2f = cst.tile([NCH, 1], dt)
            nc.vector.tensor_tensor(b2f[:], pb[:], cb2s[:], op=AL.add)

            # =========================================================
            # conv2: U2 = relu(w2f.T conv U1 + b2f)
            for t in range(12):
                pt = ps.tile([NCH, 512], dt, tag="cps")
                for hk in range(6):
                    h, k = hk // 3, hk % 3
                    wsrc = w2fh if h == 0 else w2fl
                    nc.tensor.matmul(pt[:], wsrc[:, 64 * k:64 * k + 64],
                                     U1[:, (8 * t + k) * B:(8 * t + k) * B + 512],
                                     start=(hk == 0), stop=(hk == 5))
                nc.scalar.activation(U2[:, 512 * t:512 * t + 512], pt[:],
                                     AF.Relu, bias=b2f[:], accum_out=s2sum[:, t:t + 1])
                sc = scrp.tile([128, 512], dt, tag="scr", name="sc")
                nc.scalar.activation(sc[0:NCH, :], U2[:, 512 * t:512 * t + 512],
                                     AF.Square, accum_out=s2sq[:, t:t + 1])

            bn2 = bn_vec("bn2", NCH, 1)
            bn_from_sums(bn2, s2sum, s2sq, 12, float(L2 * B), g2s[:], be2s[:])

            w3f = cst.tile([NCH, 3 * NCH], dt)
            nc.vector.tensor_scalar_mul(w3f[:], w3s[:], bn2["s"][:])
            w3fh = cst.tile([NCH, 3 * NCH], bf)
            nc.vector.tensor_copy(w3fh[:], w3f[:])
            w3fh32 = cst.tile([NCH, 3 * NCH], dt)
            nc.vector.tensor_copy(w3fh32[:], w3fh[:])
            w3fl = cst.tile([NCH, 3 * NCH], bf)
            nc.vector.tensor_tensor(w3fl[:], w3f[:], w3fh32[:], op=AL.subtract)
            pb3 = ps.tile([NCH, 1], dt, tag="cpsb")
            for k in range(3):
                nc.tensor.matmul(pb3[:], w3s[:, 64 * k:64 * k + 64], bn2["t"][:],
                                 start=(k == 0), stop=(k == 2))
            b3f = cst.tile([NCH, 1], dt)
            nc.vector.tensor_tensor(b3f[:], pb3[:], cb3s[:], op=AL.add)
            b3d = cst.tile([128, 1], dt)
            nc.vector.tensor_copy(b3d[0:NCH, :], b3f[:])
            nc.vector.tensor_copy(b3d[NCH:128, :], b3f[:])

            # =========================================================
            # conv3 (parity-packed): U3[par*64+c, l2*64+b] = relu(conv3')
            U2v = U2[:].rearrange("p (l two b) -> p two l b", two=2, b=B)
            t3l2 = [8, 8, 8, 8, 8, 7]   # 47 l2 positions
            for t in range(6):
                lw = t3l2[t]
                pt = ps.tile([128, 512], dt, tag="cps3")
                for par in range(2):
                    for hk in range(6):
                        h, k = hk // 3, hk % 3
                        wsrc = w3fh if h == 0 else w3fl
                        pk = par + k
                        rhs = U2v[:, pk % 2, 8 * t + pk // 2: 8 * t + pk // 2 + lw, :]
                        nc.tensor.matmul(pt[64 * par:64 * par + 64, 0:64 * lw],
                                         wsrc[:, 64 * k:64 * k + 64], rhs,
                                         start=(hk == 0), stop=(hk == 5),
                                         tile_position=(0, 64 * par))
                nc.scalar.activation(U3[:, 512 * t:512 * t + 64 * lw], pt[:, 0:64 * lw],
                                     AF.Relu, bias=b3d[:], accum_out=s3sum[:, t:t + 1])
                sc = scrp.tile([128, 512], dt, tag="scr")
                nc.scalar.activation(sc[:, 0:64 * lw], U3[:, 512 * t:512 * t + 64 * lw],
                                     AF.Square, accum_out=s3sq[:, t:t + 1])

            # bn3: combine parity halves, then broadcast back to 128 partitions
            S3 = cst.tile([128, 1], dt); Q3 = cst.tile([128, 1], dt)
            nc.vector.reduce_sum(S3[:], s3sum[:, 0:6], axis=mybir.AxisListType.X)
            nc.vector.reduce_sum(Q3[:], s3sq[:, 0:6], axis=mybir.AxisListType.X)
            cS = cst.tile([NCH, 1], dt); cQ = cst.tile([NCH, 1], dt)
            nc.vector.tensor_copy(cS[:], S3[NCH:128, :])
            nc.vector.tensor_copy(cQ[:], Q3[NCH:128, :])
            St = cst.tile([NCH, 1], dt); Qt = cst.tile([NCH, 1], dt)
            nc.vector.tensor_tensor(St[:], S3[0:NCH, :], cS[:], op=AL.add)
            nc.vector.tensor_tensor(Qt[:], Q3[0:NCH, :], cQ[:], op=AL.add)

            bn3 = bn_vec("bn3", NCH, 1)
            inv3 = 1.0 / float(H)
            nc.vector.tensor_scalar_mul(bn3["m"][:], St[:], inv3)
            nc.vector.tensor_scalar_mul(bn3["mq"][:], Qt[:], inv3)
            nc.vector.tensor_tensor(bn3["tmp"][:], bn3["m"][:], bn3["m"][:], op=AL.mult)
            nc.vector.tensor_tensor(bn3["var"][:], bn3["mq"][:], bn3["tmp"][:], op=AL.subtract)
            nc.scalar.activation(bn3["std"][:], bn3["var"][:], AF.Sqrt, bias=epsb[0:NCH, :])
            nc.vector.reciprocal(bn3["rstd"][:], bn3["std"][:])
            nc.vector.tensor_tensor(bn3["s"][:], g3s[:], bn3["rstd"][:], op=AL.mult)
            nc.vector.tensor_tensor(bn3["tmp"][:], bn3["m"][:], bn3["s"][:], op=AL.mult)
            nc.vector.tensor_tensor(bn3["t"][:], be3s[:], bn3["tmp"][:], op=AL.subtract)
            s3b = cst.tile([128, 1], dt); t3b = cst.tile([128, 1], dt)
            nc.vector.tensor_copy(s3b[0:NCH, :], bn3["s"][:])
            nc.vector.tensor_copy(s3b[NCH:128, :], bn3["s"][:])
            nc.vector.tensor_copy(t3b[0:NCH, :], bn3["t"][:])
            nc.vector.tensor_copy(t3b[NCH:128, :], bn3["t"][:])
            nc.vector.tensor_scalar(U3b[:], U3[:], s3b[:], t3b[:],
                                    op0=AL.mult, op1=AL.add)
            ps_cm.__exit__(None, None, None)

            psA_cm = tc.tile_pool(name="psacc", bufs=1, space="PSUM")
            psA = psA_cm.__enter__()
            psT_cm = tc.tile_pool(name="pstp", bufs=2, space="PSUM")
            psT = psT_cm.__enter__()

            # =========================================================
            # fc1 (weights moving): psum[b, f] += U3b_kk.T @ W1T_kk
            acc = [psA.tile([128, 512], dt, tag=f"acc{i}", name=f"acc{i}")
                   for i in range(3)]
            kk = 0
            for gn in G1:
                Wg = wsp.tile([128, 8 * PREAL], bf, tag="wst", name="Wg")
                nc.sync.dma_start(Wg[:, 0:gn * PREAL],
                                  W1T[:, kk * PREAL:(kk + gn) * PREAL])
                for kl in range(gn):
                    for i, (f0, fw) in enumerate(FT):
                        nc.tensor.matmul(acc[i][0:B, 0:fw],
                                         U3b[:, B * kk:B * kk + B],
                                         Wg[:, kl * PREAL + f0:kl * PREAL + f0 + fw],
                                         start=(kk == 0), stop=(kk == K1 - 1))
                    kk += 1
            for i, (f0, fw) in enumerate(FT):
                nc.vector.tensor_copy(h1s[:, f0:f0 + fw], acc[i][0:B, 0:fw])

            # transpose each 125-chunk back to [feat, batch]; relu+bias+stats
            def bn_feat(pref, sums, sqs, g_ap, b_ap):
                d = bn_vec(pref, 125, 10)
                inv = 1.0 / float(B)
                nc.vector.tensor_scalar_mul(d["m"][:], sums[:], inv)
                nc.vector.tensor_scalar_mul(d["mq"][:], sqs[:], inv)
                nc.vector.tensor_tensor(d["tmp"][:], d["m"][:], d["m"][:], op=AL.mult)
                nc.vector.tensor_tensor(d["var"][:], d["mq"][:], d["tmp"][:], op=AL.subtract)
                nc.scalar.activation(d["std"][:], d["var"][:], AF.Sqrt,
                                     bias=epsb[0:125, :])
                nc.vector.reciprocal(d["rstd"][:], d["std"][:])
                nc.vector.tensor_tensor(d["s"][:], g_ap, d["rstd"][:], op=AL.mult)
                nc.vector.tensor_tensor(d["tmp"][:], d["m"][:], d["s"][:], op=AL.mult)
                nc.vector.tensor_tensor(d["t"][:], b_ap, d["tmp"][:], op=AL.subtract)
                return d

            for c2 in range(10):
                tp = psT.tile([125, 64], dt, tag="tp", name="tp")
                nc.tensor.transpose(tp[:], h1s[:, 125 * c2:125 * c2 + 125], EYEs[:])
                nc.scalar.activation(h1[:, B * c2:B * c2 + B], tp[:],
                                     AF.Relu, bias=fb1s[:, c2:c2 + 1],
                                     accum_out=h1sum[:, c2:c2 + 1])
                sc = scrp.tile([128, 512], dt, tag="scr", name="sc")
                nc.scalar.activation(sc[0:125, 0:B], h1[:, B * c2:B * c2 + B],
                                     AF.Square, accum_out=h1sq[:, c2:c2 + 1])

            bn4 = bn_feat("bn4", h1sum, h1sq, g4s[:], be4s[:])
            for c2 in range(10):
                nc.vector.tensor_scalar(h1b[:, B * c2:B * c2 + B],
                                        h1[:, B * c2:B * c2 + B],
                                        bn4["s"][:, c2:c2 + 1], bn4["t"][:, c2:c2 + 1],
                                        op0=AL.mult, op1=AL.add)

            # =========================================================
            # AllGather the local bn4 output (125 x 640 bf16 = 160 KB)
            agin = dram.tile([125, 10 * B], bf)
            agout = dram.tile([N_CORES * 125, 10 * B], bf, addr_space="Shared")
            nc.scalar.dma_start(agin[:], h1b[:])
            if no_cc:
                for r in range(N_CORES):
                    nc.scalar.dma_start(agout[125 * r:125 * (r + 1), :], agin[:])
            else:
                nc.gpsimd.collective_compute(
                    "AllGather", mybir.AluOpType.bypass,
                    replica_groups=[list(range(N_CORES))],
                    ins=[agin[:]], outs=[agout[:]])
            nc.scalar.dma_start(
                H1B[:].rearrange("p (r n) -> p r n", r=N_CORES),
                agout[:].rearrange("(r p) n -> p r n", p=125))

            # =========================================================
            # fc2 (weights moving, row-sharded): full 10000 contraction
            acc2 = [psA.tile([128, 512], dt, tag=f"acc{i}", name=f"acc2_{i}")
                    for i in range(3)]
            for g in range(G2N):
                Wg = wsp.tile([128, 8 * PREAL], bf, tag="wst", name="Wg2")
                nc.sync.dma_start(Wg[0:125, :],
                                  W2T[:, g * 8 * PREAL:(g + 1) * 8 * PREAL])
                for jl in range(8):
                    j = 8 * g + jl
                    for i, (f0, fw) in enumerate(FT):
                        nc.tensor.matmul(acc2[i][0:B, 0:fw],
                                         H1B[:, B * j:B * j + B],
                                         Wg[0:125, jl * PREAL + f0:jl * PREAL + f0 + fw],
                                         start=(j == 0), stop=(j == J2 - 1))
            for i, (f0, fw) in enumerate(FT):
                nc.vector.tensor_copy(h2s[:, f0:f0 + fw], acc2[i][0:B, 0:fw])

            for c2 in range(10):
                tp = psT.tile([125, 64], dt, tag="tp", name="tp2")
                nc.tensor.transpose(tp[:], h2s[:, 125 * c2:125 * c2 + 125], EYEs[:])
                nc.scalar.activation(h2r[:, B * c2:B * c2 + B], tp[:],
                                     AF.Relu, bias=fb2s[:, c2:c2 + 1],
                                     accum_out=h2sum[:, c2:c2 + 1])
                sc = scrp.tile([128, 512], dt, tag="scr", name="sc")
                nc.scalar.activation(sc[0:125, 0:B], h2r[:, B * c2:B * c2 + B],
                                     AF.Square, accum_out=h2sq[:, c2:c2 + 1])

            bn5 = bn_feat("bn5", h2sum, h2sq, g5s[:], be5s[:])
            for c2 in range(10):
                nc.vector.tensor_scalar(h2b[:, B * c2:B * c2 + B],
                                        h2r[:, B * c2:B * c2 + B],
                                        bn5["s"][:, c2:c2 + 1], bn5["t"][:, c2:c2 + 1],
                                        op0=AL.mult, op1=AL.add)

            # =========================================================
            # fc3 (weights moving, contraction-sharded; epilogue in weights)
            acc3 = [psA.tile([128, 512], dt, tag=f"acc{i}", name=f"acc3_{i}")
                    for i in range(3)]
            for c2 in range(10):
                for i, (f0, fw) in enumerate(FT3):
                    nc.tensor.matmul(acc3[i][0:B, 0:fw],
                                     h2b[:, B * c2:B * c2 + B],
                                     W3s[:, c2 * OUTF + f0:c2 * OUTF + f0 + fw],
                                     start=(c2 == 0), stop=(c2 == 9))
            for i, (f0, fw) in enumerate(FT3):
                nc.vector.tensor_copy(q3[:, f0:f0 + fw], acc3[i][0:B, 0:fw])

            nc.scalar.dma_start(out[:], q3[:])
            psT_cm.__exit__(None, None, None)
            psA_cm.__exit__(None, None, None)

    nc.compile()
    return nc


# ---------------------------------------------------------------- entry point

def _run_sim(nc, in_maps):
    from concourse.bass_interp import MultiCoreSim

    sim = MultiCoreSim(nc, num_cores=N_CORES, trace=False,
                       require_finite=False, require_nnan=False)
    for i, (cid, core) in enumerate(sim.cores.items()):
        for name, arr in in_maps[i].items():
            core.tensor(name)[:] = arr
    sim.simulate(check_with_hw=False)
    return [np.array(sim.cores[c].tensor("out")) for c in range(N_CORES)]


def _finish(outs, b3e):
    acc = np.zeros((B, OUTF), np.float64)
    for o in outs:
        acc += np.asarray(o, np.float64)
    acc += b3e.astype(np.float64)[None, :]
    return np.ascontiguousarray(acc).reshape(B, 12, 100).astype(np.float32)


def kernel(**inputs):
    from concourse import bass_utils

    if "nc" not in _CACHE:
        _CACHE["nc"] = _build_nc()
    nc = _CACHE["nc"]

    in_maps, b3e = _prep_in_maps(inputs)
    outs = None
    for attempt in range(2):
        try:
            res = bass_utils.run_bass_kernel_spmd(
                nc, in_maps, core_ids=list(range(N_CORES)))
            outs = [res.results[i]["out"] for i in range(N_CORES)]
            print(f"[kernel] hw run ok (attempt {attempt})", file=sys.stderr)
            break
        except Exception as e:
            # device may be wedged from a prior run; one retry usually
            # recovers it. After that, fall back to the simulator.
            print(f"[kernel] hw attempt {attempt} failed: {type(e).__name__}",
                  file=sys.stderr)
            continue
    if outs is None:
        print("[kernel] falling back to simulator", file=sys.stderr)
        outs = _run_sim(nc, in_maps)
    return _finish(outs, b3e)


# revision 19
# speedup vs baseline: 2.1953x; 1.0489x over previous
"""Trainium2 Bass kernel for nn_Conv1dMapper (3x conv1d+bn -> 3x fc+bn -> interp epilogue).

Self-contained: accepts FULL inputs, shards across 8 NeuronCores internally,
returns the FULL [64, 12, 100] output.

Sharding strategy (v3, weights-moving):
  - conv stage (tiny) replicated on all cores in bf16; bn1/bn2 folded into the
    next conv's weights at runtime; conv3 packs even/odd output positions into
    128 partitions via PE column-tiling so fc1 gets K=128 contraction chunks.
  - ALL fc matmuls are "weights-moving": the activations (batch=64 wide) are
    the stationary operand and the weights stream through the PE array with a
    512-wide free dim.  This turns each fc layer into ~N_chunks*3 large
    matmuls instead of thousands of N=64 ones.
  - fc1 row-sharded (1250 rows/core).  Output lands as [batch, feat] in PSUM;
    PE-transpose (identity matmul) flips each 125-chunk back to [feat, batch],
    where bias+relu+bn4 run exactly like a weights-stationary kernel.
  - the bf16 bn4 output (160 KB) is AllGather'd so every core holds the full
    10000-dim h1; fc2 is then row-sharded too (1250 outs/core, full
    contraction) - no ReduceScatter of fp32 partials needed.
  - fc3 contraction-sharded with the output epilogue folded into its weights;
    fp32 partials [64, 1200] are returned per-core and summed on the host.
"""

import sys

sys.path.insert(0, "/opt/trn_rl_repo")

import numpy as np

N_CORES = 8
B = 64            # batch
L1, L2, L3 = 98, 96, 94
NCH = 64          # conv channels
H = 6016          # fc1 in features = 64*94
L3H = 47          # = L3 // 2
HID = 10000
PREAL = 1250      # fc1/fc2 output rows per core
EPS = 1e-5
OUTF = 1200

K1 = 47           # fc1 contraction chunks (128 each)
J2 = 80           # fc2 contraction chunks (125 each)
FT = [(0, 512), (512, 512), (1024, 226)]    # f-tiles over 1250 local outs
FT3 = [(0, 512), (512, 512), (1024, 176)]   # f-tiles over 1200 outs
G1 = [4] * 11 + [3]                         # fc1 kk-chunk DMA groups
G2N = 20                                    # fc2 j-chunk DMA groups (4 each)

_CACHE = {}


# ---------------------------------------------------------------- host prep

def _fold_epilogue(fc3_w, fc3_b):
    """Fold reshape->zero/one channels->interpolation into fc3's weights."""
    L = 100
    CD = L // 3
    CPS = np.array([1, CD, 2 * CD, 3 * CD])
    REG = np.array([p for p in range(L) if p not in set(CPS.tolist())][1:])
    J = REG // CD
    Lp = CPS[J]
    Rp = CPS[J + 1]
    ALPHA = ((REG - Lp) / CD).astype(np.float32)
    CH = np.array([0, 2, 8, 10, 3, 11])
    ZERO_CH = np.array([1, 4, 6, 7, 9])

    W3e = fc3_w.astype(np.float32).copy()
    b3e = fc3_b.astype(np.float32).copy()
    idx0 = (ZERO_CH[:, None] * L + np.arange(L)[None, :]).ravel()
    W3e[idx0] = 0.0
    b3e[idx0] = 0.0
    idx1 = 5 * L + np.arange(L)
    W3e[idx1] = 0.0
    b3e[idx1] = 1.0
    rows_t = (CH[:, None] * L + REG[None, :]).ravel()
    rows_l = (CH[:, None] * L + Lp[None, :]).ravel()
    rows_r = (CH[:, None] * L + Rp[None, :]).ravel()
    a = np.broadcast_to(ALPHA[None, :], (len(CH), len(REG))).ravel()[:, None]
    W3e[rows_t] = a * fc3_w[rows_l] + (1.0 - a) * fc3_w[rows_r]
    b3e[rows_t] = (a[:, 0] * fc3_b[rows_l] + (1.0 - a[:, 0]) * fc3_b[rows_r])
    return W3e, b3e


def _prep_in_maps(inp):
    from concourse import mybir

    f32 = np.float32
    bf16 = mybir.dt.np(mybir.dt.bfloat16)
    x = np.asarray(inp["x"], f32)

    # conv1 im2col: X9[k*3+i, l*64+b] = x[b, i, l+k]
    x_t = np.ascontiguousarray(x.transpose(1, 2, 0))      # [3, 100, 64]
    X9 = np.stack([x_t[:, k:k + L1, :] for k in range(3)], 0)  # [k, i, l, b]
    X9 = X9.reshape(9, L1 * B)
    X9 = np.ascontiguousarray(
        np.concatenate([X9, np.zeros((23, L1 * B), f32)], 0)).astype(bf16)

    w1 = np.asarray(inp["conv1_w"], f32).transpose(2, 1, 0).reshape(9, NCH)
    w1 = np.ascontiguousarray(
        np.concatenate([w1, np.zeros((23, NCH), f32)], 0))
    w1h = w1.astype(bf16)
    w1l = (w1 - w1h.astype(f32)).astype(bf16)
    w2 = np.ascontiguousarray(
        np.asarray(inp["conv2_w"], f32).transpose(1, 2, 0).reshape(NCH, 3 * NCH))
    w3 = np.ascontiguousarray(
        np.asarray(inp["conv3_w"], f32).transpose(1, 2, 0).reshape(NCH, 3 * NCH))

    common = {
        "X9": X9, "w1h": w1h, "w1l": w1l, "w2": w2, "w3": w3,
        "cb1": np.asarray(inp["conv1_b"], f32),
        "cb2": np.asarray(inp["conv2_b"], f32),
        "cb3": np.asarray(inp["conv3_b"], f32),
        "g1": np.asarray(inp["bn1_g"], f32), "be1": np.asarray(inp["bn1_b"], f32),
        "g2": np.asarray(inp["bn2_g"], f32), "be2": np.asarray(inp["bn2_b"], f32),
        "g3": np.asarray(inp["bn3_g"], f32), "be3": np.asarray(inp["bn3_b"], f32),
        "EYE": np.eye(B, dtype=f32),
    }

    fc1_w = np.asarray(inp["fc1_w"], f32)
    fc2_w = np.asarray(inp["fc2_w"], f32)
    W3e, b3e = _fold_epilogue(np.asarray(inp["fc3_w"], f32),
                              np.asarray(inp["fc3_b"], f32))

    def tiles(vec, p, n):  # [p*n] -> [p, n] with t[j, c] = vec[c*p+j]
        return np.ascontiguousarray(np.asarray(vec, f32).reshape(n, p).T)

    in_maps = []
    for c in range(N_CORES):
        m = dict(common)
        r0 = PREAL * c

        # ---- fc1 shard: W1T[p, kk*1250 + f] = fc1_w[r0+f, feat(p,kk)]
        # feat = ch*94 + 2*kk + par with p = par*64 + ch  (conv3 parity pack)
        A = fc1_w[r0:r0 + PREAL]                       # [1250, 6016]
        A6 = A.reshape(PREAL, NCH, L3H, 2)             # [f, ch, kk, par]
        W1T = A6.transpose(3, 1, 2, 0).reshape(128, L3H, PREAL)
        m["W1T"] = np.ascontiguousarray(
            W1T.reshape(128, L3H * PREAL)).astype(bf16)
        m["fb1"] = tiles(np.asarray(inp["fc1_b"], f32)[r0:r0 + PREAL], 125, 10)
        m["g4"] = tiles(np.asarray(inp["bn4_g"], f32)[r0:r0 + PREAL], 125, 10)
        m["be4"] = tiles(np.asarray(inp["bn4_b"], f32)[r0:r0 + PREAL], 125, 10)

        # ---- fc2 shard (row): W2T[p, j*1250 + f] = fc2_w[r0+f, j*125+p]
        Bc = fc2_w[r0:r0 + PREAL, :]                   # [1250 outs, 10000 in]
        V = Bc.T.reshape(J2, 125, PREAL)               # [j, p, f]
        m["W2T"] = np.ascontiguousarray(
            V.transpose(1, 0, 2).reshape(125, J2 * PREAL)).astype(bf16)
        m["fb2"] = tiles(np.asarray(inp["fc2_b"], f32)[r0:r0 + PREAL], 125, 10)
        m["g5"] = tiles(np.asarray(inp["bn5_g"], f32)[r0:r0 + PREAL], 125, 10)
        m["be5"] = tiles(np.asarray(inp["bn5_b"], f32)[r0:r0 + PREAL], 125, 10)

        # ---- fc3 contraction shard: W3T[p, c2*1200 + f] = W3e[f, r0+c2*125+p]
        C = W3e[:, r0:r0 + PREAL]                      # [1200, 1250]
        V3 = C.T.reshape(10, 125, OUTF)                # [c2, p, f]
        m["W3T"] = np.ascontiguousarray(
            V3.transpose(1, 0, 2).reshape(125, 10 * OUTF)).astype(bf16)
        in_maps.append(m)
    return in_maps, b3e


# ---------------------------------------------------------------- device build

def _build_nc(no_cc=False):
    from concourse import bacc, tile, mybir

    dt = mybir.dt.float32
    bf = mybir.dt.bfloat16
    AF = mybir.ActivationFunctionType
    AL = mybir.AluOpType

    nc = bacc.Bacc("TRN2", target_bir_lowering=False, debug=False,
                   num_devices=N_CORES)

    def din(name, shape, d=dt):
        return nc.dram_tensor(name, list(shape), d, kind="ExternalInput").ap()

    X9 = din("X9", [32, L1 * B], bf)
    w1h = din("w1h", [32, NCH], bf); w1l = din("w1l", [32, NCH], bf)
    cb1 = din("cb1", [NCH])
    w2 = din("w2", [NCH, 3 * NCH]); cb2 = din("cb2", [NCH])
    w3 = din("w3", [NCH, 3 * NCH]); cb3 = din("cb3", [NCH])
    g1 = din("g1", [NCH]); be1 = din("be1", [NCH])
    g2 = din("g2", [NCH]); be2 = din("be2", [NCH])
    g3 = din("g3", [NCH]); be3 = din("be3", [NCH])
    EYE = din("EYE", [B, B])
    W1T = din("W1T", [128, K1 * PREAL], bf)
    fb1 = din("fb1", [125, 10]); g4 = din("g4", [125, 10]); be4 = din("be4", [125, 10])
    W2T = din("W2T", [125, J2 * PREAL], bf)
    fb2 = din("fb2", [125, 10]); g5 = din("g5", [125, 10]); be5 = din("be5", [125, 10])
    W3T = din("W3T", [125, 10 * OUTF], bf)
    out = nc.dram_tensor("out", [B, OUTF], dt, kind="ExternalOutput").ap()

    with tile.TileContext(nc) as tc:
        with (tc.tile_pool(name="const", bufs=1) as cst,
              tc.tile_pool(name="acts", bufs=1) as acts,
              tc.tile_pool(name="wsp", bufs=6) as wsp,
              tc.tile_pool(name="w3p", bufs=1) as w3p,
              tc.tile_pool(name="scr", bufs=2) as scrp,
              tc.tile_pool(name="dram", bufs=1, space="DRAM") as dram):

            # ---- load constants / small tensors
            X9s = cst.tile([32, L1 * B], bf)
            nc.sync.dma_start(X9s[:], X9[:])
            w1hs = cst.tile([32, NCH], bf); nc.sync.dma_start(w1hs[:], w1h[:])
            w1ls = cst.tile([32, NCH], bf); nc.sync.dma_start(w1ls[:], w1l[:])
            w2s = cst.tile([NCH, 3 * NCH], dt); nc.sync.dma_start(w2s[:], w2[:])
            w3s = cst.tile([NCH, 3 * NCH], dt); nc.sync.dma_start(w3s[:], w3[:])
            EYEs = cst.tile([B, B], dt); nc.sync.dma_start(EYEs[:], EYE[:])

            def vec64(ap):
                t = cst.tile([NCH, 1], dt, tag=f"v64_{ap.name}")
                nc.sync.dma_start(t[:], ap[:, None])
                return t
            cb1s, cb2s, cb3s = vec64(cb1), vec64(cb2), vec64(cb3)
            g1s, be1s = vec64(g1), vec64(be1)
            g2s, be2s = vec64(g2), vec64(be2)
            g3s, be3s = vec64(g3), vec64(be3)

            def mat(ap, p, n):
                t = cst.tile([p, n], dt, tag=f"m_{ap.name}")
                nc.sync.dma_start(t[:], ap[:])
                return t
            fb1s, g4s, be4s = mat(fb1, 125, 10), mat(g4, 125, 10), mat(be4, 125, 10)
            fb2s, g5s, be5s = mat(fb2, 125, 10), mat(g5, 125, 10), mat(be5, 125, 10)
            epsb = cst.tile([128, 1], dt, name="epsb")
            nc.vector.memset(epsb[:], EPS)

            # fc3 weights (loaded after the fc1 stream, see below)
            W3s = w3p.tile([125, 10 * OUTF], bf, name="W3s")

            # ---- activations
            U1 = acts.tile([NCH, L1 * B], bf)       # relu(conv1+b)
            U2 = acts.tile([NCH, L2 * B], bf)       # relu(conv2'+b2')
            U3 = acts.tile([128, L3H * B], dt)      # relu(conv3'+b3'), parity-packed
            U3b = acts.tile([128, L3H * B], bf)     # bn3 applied, bf16 for fc1
            h1s = acts.tile([B, PREAL], dt)         # fc1 raw out [batch, feat]
            h1 = acts.tile([125, 10 * B], dt)       # fc1 relu out [feat, batch]
            h1b = acts.tile([125, 10 * B], bf)      # bn4 out, bf16
            H1B = acts.tile([125, J2 * B], bf)      # AllGather'd full h1 (10000)
            h2s = acts.tile([B, PREAL], dt)         # fc2 raw out [batch, feat]
            h2r = acts.tile([125, 10 * B], dt)      # relu(fc2+b)
            h2b = acts.tile([125, 10 * B], bf)      # bn5 out
            q3 = acts.tile([B, OUTF], dt)           # fc3 partials

            # stats tiles
            s1sum = cst.tile([NCH, 16], dt); s1sq = cst.tile([NCH, 16], dt)
            s2sum = cst.tile([NCH, 16], dt); s2sq = cst.tile([NCH, 16], dt)
            s3sum = cst.tile([128, 8], dt); s3sq = cst.tile([128, 8], dt)
            h1sum = cst.tile([125, 10], dt); h1sq = cst.tile([125, 10], dt)
            h2sum = cst.tile([125, 10], dt); h2sq = cst.tile([125, 10], dt)

            def bn_vec(pref, p, n):
                return {k: cst.tile([p, n], dt, tag=f"{pref}_{k}",
                                    name=f"{pref}_{k}")
                        for k in ("S", "Q", "m", "mq", "var", "std", "rstd",
                                  "s", "t", "tmp")}

            def bn_from_sums(d, sums, sqs, ntile, count, g_ap, b_ap):
                """Per-partition bn scale/shift from per-tile sums."""
                nc.vector.reduce_sum(d["S"][:], sums[:, 0:ntile], axis=mybir.AxisListType.X)
                nc.vector.reduce_sum(d["Q"][:], sqs[:, 0:ntile], axis=mybir.AxisListType.X)
                inv = 1.0 / count
                nc.vector.tensor_scalar_mul(d["m"][:], d["S"][:], inv)
                nc.vector.tensor_scalar_mul(d["mq"][:], d["Q"][:], inv)
                nc.vector.tensor_tensor(d["tmp"][:], d["m"][:], d["m"][:], op=AL.mult)
                nc.vector.tensor_tensor(d["var"][:], d["mq"][:], d["tmp"][:], op=AL.subtract)
                nc.scalar.activation(d["std"][:], d["var"][:], AF.Sqrt,
                                     bias=epsb[0:d["var"].shape[0], :])
                nc.vector.reciprocal(d["rstd"][:], d["std"][:])
                nc.vector.tensor_tensor(d["s"][:], g_ap, d["rstd"][:], op=AL.mult)
                nc.vector.tensor_tensor(d["tmp"][:], d["m"][:], d["s"][:], op=AL.mult)
                nc.vector.tensor_tensor(d["t"][:], b_ap, d["tmp"][:], op=AL.subtract)

            # =========================================================
            # conv1: U1 = relu(w1.T @ X9 + cb1)
            ps_cm = tc.tile_pool(name="pscv", bufs=2, space="PSUM")
            ps = ps_cm.__enter__()
            n1 = L1 * B  # 6272
            t1sizes = [512] * 12 + [128]
            for t in range(13):
                sz = t1sizes[t]
                pt = ps.tile([NCH, 512], dt, tag="cps")
                nc.tensor.matmul(pt[:, 0:sz], w1hs[:],
                                 X9s[:, 512 * t:512 * t + sz],
                                 start=True, stop=False)
                nc.tensor.matmul(pt[:, 0:sz], w1ls[:],
                                 X9s[:, 512 * t:512 * t + sz],
                                 start=False, stop=True)
                nc.scalar.activation(U1[:, 512 * t:512 * t + sz], pt[:, 0:sz],
                                     AF.Relu, bias=cb1s[:], accum_out=s1sum[:, t:t + 1])
                sc = scrp.tile([128, 512], dt, tag="scr", name="sc")
                nc.scalar.activation(sc[0:NCH, 0:sz], U1[:, 512 * t:512 * t + sz],
                                     AF.Square, accum_out=s1sq[:, t:t + 1])

            bn1 = bn_vec("bn1", NCH, 1)
            bn_from_sums(bn1, s1sum, s1sq, 13, float(n1), g1s[:], be1s[:])

            # fold bn1 into conv2 weights: w2f = w2 * s1 (per in-channel),
            # b2f = cb2 + sum_k w2[k].T @ t1
            w2f = cst.tile([NCH, 3 * NCH], dt)
            nc.vector.tensor_scalar_mul(w2f[:], w2s[:], bn1["s"][:])
            w2fh = cst.tile([NCH, 3 * NCH], bf)
            nc.vector.tensor_copy(w2fh[:], w2f[:])
            pb = ps.tile([NCH, 1], dt, tag="cpsb")
            for k in range(3):
                nc.tensor.matmul(pb[:], w2s[:, 64 * k:64 * k + 64], bn1["t"][:],
                                 start=(k == 0), stop=(k == 2))
            b2f = cst.tile([NCH, 1], dt)
            nc.vector.tensor_tensor(b2f[:], pb[:], cb2s[:], op=AL.add)

            # =========================================================
            # conv2: U2 = relu(w2f.T conv U1 + b2f)
            for t in range(12):
                pt = ps.tile([NCH, 512], dt, tag="cps")
                for k in range(3):
                    nc.tensor.matmul(pt[:], w2fh[:, 64 * k:64 * k + 64],
                                     U1[:, (8 * t + k) * B:(8 * t + k) * B + 512],
                                     start=(k == 0), stop=(k == 2))
                nc.scalar.activation(U2[:, 512 * t:512 * t + 512], pt[:],
                                     AF.Relu, bias=b2f[:], accum_out=s2sum[:, t:t + 1])
                sc = scrp.tile([128, 512], dt, tag="scr", name="sc")
                nc.scalar.activation(sc[0:NCH, :], U2[:, 512 * t:512 * t + 512],
                                     AF.Square, accum_out=s2sq[:, t:t + 1])

            bn2 = bn_vec("bn2", NCH, 1)
            bn_from_sums(bn2, s2sum, s2sq, 12, float(L2 * B), g2s[:], be2s[:])

            w3f = cst.tile([NCH, 3 * NCH], dt)
            nc.vector.tensor_scalar_mul(w3f[:], w3s[:], bn2["s"][:])
            w3fh = cst.tile([NCH, 3 * NCH], bf)
            nc.vector.tensor_copy(w3fh[:], w3f[:])
            pb3 = ps.tile([NCH, 1], dt, tag="cpsb")
            for k in range(3):
                nc.tensor.matmul(pb3[:], w3s[:, 64 * k:64 * k + 64], bn2["t"][:],
                                 start=(k == 0), stop=(k == 2))
            b3f = cst.tile([NCH, 1], dt)
            nc.vector.tensor_tensor(b3f[:], pb3[:], cb3s[:], op=AL.add)
            b3d = cst.tile([128, 1], dt)
            nc.vector.tensor_copy(b3d[0:NCH, :], b3f[:])
            nc.vector.tensor_copy(b3d[NCH:128, :], b3f[:])

            # =========================================================
            # conv3 (parity-packed): U3[par*64+c, l2*64+b] = relu(conv3')
            U2v = U2[:].rearrange("p (l two b) -> p two l b", two=2, b=B)
            t3l2 = [8, 8, 8, 8, 8, 7]   # 47 l2 positions
            for t in range(6):
                lw = t3l2[t]
                pt = ps.tile([128, 512], dt, tag="cps3")
                for par in range(2):
                    for k in range(3):
                        pk = par + k
                        rhs = U2v[:, pk % 2, 8 * t + pk // 2: 8 * t + pk // 2 + lw, :]
                        nc.tensor.matmul(pt[64 * par:64 * par + 64, 0:64 * lw],
                                         w3fh[:, 64 * k:64 * k + 64], rhs,
                                         start=(k == 0), stop=(k == 2),
                                         tile_position=(0, 64 * par))
                nc.scalar.activation(U3[:, 512 * t:512 * t + 64 * lw], pt[:, 0:64 * lw],
                                     AF.Relu, bias=b3d[:], accum_out=s3sum[:, t:t + 1])
                sc = scrp.tile([128, 512], dt, tag="scr")
                nc.scalar.activation(sc[:, 0:64 * lw], U3[:, 512 * t:512 * t + 64 * lw],
                                     AF.Square, accum_out=s3sq[:, t:t + 1])

            # bn3: combine parity halves, then broadcast back to 128 partitions
            S3 = cst.tile([128, 1], dt); Q3 = cst.tile([128, 1], dt)
            nc.vector.reduce_sum(S3[:], s3sum[:, 0:6], axis=mybir.AxisListType.X)
            nc.vector.reduce_sum(Q3[:], s3sq[:, 0:6], axis=mybir.AxisListType.X)
            cS = cst.tile([NCH, 1], dt); cQ = cst.tile([NCH, 1], dt)
            nc.vector.tensor_copy(cS[:], S3[NCH:128, :])
            nc.vector.tensor_copy(cQ[:], Q3[NCH:128, :])
            St = cst.tile([NCH, 1], dt); Qt = cst.tile([NCH, 1], dt)
            nc.vector.tensor_tensor(St[:], S3[0:NCH, :], cS[:], op=AL.add)
            nc.vector.tensor_tensor(Qt[:], Q3[0:NCH, :], cQ[:], op=AL.add)

            bn3 = bn_vec("bn3", NCH, 1)
            inv3 = 1.0 / float(H)
            nc.vector.tensor_scalar_mul(bn3["m"][:], St[:], inv3)
            nc.vector.tensor_scalar_mul(bn3["mq"][:], Qt[:], inv3)
            nc.vector.tensor_tensor(bn3["tmp"][:], bn3["m"][:], bn3["m"][:], op=AL.mult)
            nc.vector.tensor_tensor(bn3["var"][:], bn3["mq"][:], bn3["tmp"][:], op=AL.subtract)
            nc.scalar.activation(bn3["std"][:], bn3["var"][:], AF.Sqrt, bias=epsb[0:NCH, :])
            nc.vector.reciprocal(bn3["rstd"][:], bn3["std"][:])
            nc.vector.tensor_tensor(bn3["s"][:], g3s[:], bn3["rstd"][:], op=AL.mult)
            nc.vector.tensor_tensor(bn3["tmp"][:], bn3["m"][:], bn3["s"][:], op=AL.mult)
            nc.vector.tensor_tensor(bn3["t"][:], be3s[:], bn3["tmp"][:], op=AL.subtract)
            s3b = cst.tile([128, 1], dt); t3b = cst.tile([128, 1], dt)
            nc.vector.tensor_copy(s3b[0:NCH, :], bn3["s"][:])
            nc.vector.tensor_copy(s3b[NCH:128, :], bn3["s"][:])
            nc.vector.tensor_copy(t3b[0:NCH, :], bn3["t"][:])
            nc.vector.tensor_copy(t3b[NCH:128, :], bn3["t"][:])
            nc.vector.tensor_scalar(U3b[:], U3[:], s3b[:], t3b[:],
                                    op0=AL.mult, op1=AL.add)
            ps_cm.__exit__(None, None, None)

            psA_cm = tc.tile_pool(name="psacc", bufs=1, space="PSUM")
            psA = psA_cm.__enter__()
            psT_cm = tc.tile_pool(name="pstp", bufs=2, space="PSUM")
            psT = psT_cm.__enter__()

            # =========================================================
            # fc1 (weights moving): psum[b, f] += U3b_kk.T @ W1T_kk
            acc = [psA.tile([128, 512], dt, tag=f"acc{i}", name=f"acc{i}")
                   for i in range(3)]
            kk = 0
            for gn in G1:
                Wg = wsp.tile([128, 4 * PREAL], bf, tag="wst", name="Wg")
                nc.sync.dma_start(Wg[:, 0:gn * PREAL],
                                  W1T[:, kk * PREAL:(kk + gn) * PREAL])
                for kl in range(gn):
                    for i, (f0, fw) in enumerate(FT):
                        nc.tensor.matmul(acc[i][0:B, 0:fw],
                                         U3b[:, B * kk:B * kk + B],
                                         Wg[:, kl * PREAL + f0:kl * PREAL + f0 + fw],
                                         start=(kk == 0), stop=(kk == K1 - 1))
                    kk += 1
            nc.sync.dma_start(W3s[:], W3T[:])
            for i, (f0, fw) in enumerate(FT):
                nc.vector.tensor_copy(h1s[:, f0:f0 + fw], acc[i][0:B, 0:fw])

            # transpose each 125-chunk back to [feat, batch]; relu+bias+stats
            def bn_feat(pref, sums, sqs, g_ap, b_ap):
                d = bn_vec(pref, 125, 10)
                inv = 1.0 / float(B)
                nc.vector.tensor_scalar_mul(d["m"][:], sums[:], inv)
                nc.vector.tensor_scalar_mul(d["mq"][:], sqs[:], inv)
                nc.vector.tensor_tensor(d["tmp"][:], d["m"][:], d["m"][:], op=AL.mult)
                nc.vector.tensor_tensor(d["var"][:], d["mq"][:], d["tmp"][:], op=AL.subtract)
                nc.scalar.activation(d["std"][:], d["var"][:], AF.Sqrt,
                                     bias=epsb[0:125, :])
                nc.vector.reciprocal(d["rstd"][:], d["std"][:])
                nc.vector.tensor_tensor(d["s"][:], g_ap, d["rstd"][:], op=AL.mult)
                nc.vector.tensor_tensor(d["tmp"][:], d["m"][:], d["s"][:], op=AL.mult)
                nc.vector.tensor_tensor(d["t"][:], b_ap, d["tmp"][:], op=AL.subtract)
                return d

            for c2 in range(10):
                tp = psT.tile([125, 64], dt, tag="tp", name="tp")
                nc.tensor.transpose(tp[:], h1s[:, 125 * c2:125 * c2 + 125], EYEs[:])
                nc.scalar.activation(h1[:, B * c2:B * c2 + B], tp[:],
                                     AF.Relu, bias=fb1s[:, c2:c2 + 1],
                                     accum_out=h1sum[:, c2:c2 + 1])
                sc = scrp.tile([128, 512], dt, tag="scr", name="sc")
                nc.scalar.activation(sc[0:125, 0:B], h1[:, B * c2:B * c2 + B],
                                     AF.Square, accum_out=h1sq[:, c2:c2 + 1])

            bn4 = bn_feat("bn4", h1sum, h1sq, g4s[:], be4s[:])
            for c2 in range(10):
                nc.vector.tensor_scalar(h1b[:, B * c2:B * c2 + B],
                                        h1[:, B * c2:B * c2 + B],
                                        bn4["s"][:, c2:c2 + 1], bn4["t"][:, c2:c2 + 1],
                                        op0=AL.mult, op1=AL.add)

            # =========================================================
            # AllGather the local bn4 output (125 x 640 bf16 = 160 KB)
            agin = dram.tile([125, 10 * B], bf)
            agout = dram.tile([N_CORES * 125, 10 * B], bf, addr_space="Shared")
            nc.gpsimd.dma_start(agin[:], h1b[:])
            if no_cc:
                for r in range(N_CORES):
                    nc.gpsimd.dma_start(agout[125 * r:125 * (r + 1), :], agin[:])
            else:
                nc.gpsimd.collective_compute(
                    "AllGather", mybir.AluOpType.bypass,
                    replica_groups=[list(range(N_CORES))],
                    ins=[agin[:]], outs=[agout[:]])
            nc.gpsimd.dma_start(
                H1B[:].rearrange("p (r n) -> p r n", r=N_CORES),
                agout[:].rearrange("(r p) n -> p r n", p=125))

            # =========================================================
            # fc2 (weights moving, row-sharded): full 10000 contraction
            acc2 = [psA.tile([128, 512], dt, tag=f"acc{i}", name=f"acc2_{i}")
                    for i in range(3)]
            for g in range(G2N):
                Wg = wsp.tile([128, 4 * PREAL], bf, tag="wst", name="Wg2")
                nc.sync.dma_start(Wg[0:125, :],
                                  W2T[:, g * 4 * PREAL:(g + 1) * 4 * PREAL])
                for jl in range(4):
                    j = 4 * g + jl
                    for i, (f0, fw) in enumerate(FT):
                        nc.tensor.matmul(acc2[i][0:B, 0:fw],
                                         H1B[:, B * j:B * j + B],
                                         Wg[0:125, jl * PREAL + f0:jl * PREAL + f0 + fw],
                                         start=(j == 0), stop=(j == J2 - 1))
            for i, (f0, fw) in enumerate(FT):
                nc.vector.tensor_copy(h2s[:, f0:f0 + fw], acc2[i][0:B, 0:fw])

            for c2 in range(10):
                tp = psT.tile([125, 64], dt, tag="tp", name="tp2")
                nc.tensor.transpose(tp[:], h2s[:, 125 * c2:125 * c2 + 125], EYEs[:])
                nc.scalar.activation(h2r[:, B * c2:B * c2 + B], tp[:],
                                     AF.Relu, bias=fb2s[:, c2:c2 + 1],
                                     accum_out=h2sum[:, c2:c2 + 1])
                sc = scrp.tile([128, 512], dt, tag="scr", name="sc")
                nc.scalar.activation(sc[0:125, 0:B], h2r[:, B * c2:B * c2 + B],
                                     AF.Square, accum_out=h2sq[:, c2:c2 + 1])

            bn5 = bn_feat("bn5", h2sum, h2sq, g5s[:], be5s[:])
            for c2 in range(10):
                nc.vector.tensor_scalar(h2b[:, B * c2:B * c2 + B],
                                        h2r[:, B * c2:B * c2 + B],
                                        bn5["s"][:, c2:c2 + 1], bn5["t"][:, c2:c2 + 1],
                                        op0=AL.mult, op1=AL.add)

            # =========================================================
            # fc3 (weights moving, contraction-sharded; epilogue in weights)
            acc3 = [psA.tile([128, 512], dt, tag=f"acc{i}", name=f"acc3_{i}")
                    for i in range(3)]
            for c2 in range(10):
                for i, (f0, fw) in enumerate(FT3):
                    nc.tensor.matmul(acc3[i][0:B, 0:fw],
                                     h2b[:, B * c2:B * c2 + B],
                                     W3s[:, c2 * OUTF + f0:c2 * OUTF + f0 + fw],
                                     start=(c2 == 0), stop=(c2 == 9))
            for i, (f0, fw) in enumerate(FT3):
                nc.vector.tensor_copy(q3[:, f0:f0 + fw], acc3[i][0:B, 0:fw])

            nc.scalar.dma_start(out[:], q3[:])
            psT_cm.__exit__(None, None, None)
            psA_cm.__exit__(None, None, None)

    nc.compile()
    return nc


# ---------------------------------------------------------------- entry point

def _run_sim(nc, in_maps):
    from concourse.bass_interp import MultiCoreSim

    sim = MultiCoreSim(nc, num_cores=N_CORES, trace=False,
                       require_finite=False, require_nnan=False)
    for i, (cid, core) in enumerate(sim.cores.items()):
        for name, arr in in_maps[i].items():
            core.tensor(name)[:] = arr
    sim.simulate(check_with_hw=False)
    return [np.array(sim.cores[c].tensor("out")) for c in range(N_CORES)]


def _finish(outs, b3e):
    acc = np.zeros((B, OUTF), np.float64)
    for o in outs:
        acc += np.asarray(o, np.float64)
    acc += b3e.astype(np.float64)[None, :]
    return np.ascontiguousarray(acc).reshape(B, 12, 100).astype(np.float32)


def kernel(**inputs):
    from concourse import bass_utils

    if "nc" not in _CACHE:
        _CACHE["nc"] = _build_nc()
    nc = _CACHE["nc"]

    in_maps, b3e = _prep_in_maps(inputs)
    outs = None
    for attempt in range(2):
        try:
            res = bass_utils.run_bass_kernel_spmd(
                nc, in_maps, core_ids=list(range(N_CORES)))
            outs = [res.results[i]["out"] for i in range(N_CORES)]
            print(f"[kernel] hw run ok (attempt {attempt})", file=sys.stderr)
            break
        except Exception as e:
            # device may be wedged from a prior run; one retry usually
            # recovers it. After that, fall back to the simulator.
            print(f"[kernel] hw attempt {attempt} failed: {type(e).__name__}",
                  file=sys.stderr)
            continue
    if outs is None:
        print("[kernel] falling back to simulator", file=sys.stderr)
        outs = _run_sim(nc, in_maps)
    return _finish(outs, b3e)
